# revision 4
# baseline (speedup 1.0000x reference)
"""Distributed cosine-similarity kNN retrieval (EpisodicSDM) on 8 Trainium2 cores.

Reference computation:
    x_norm = normalize(x); k_norm = normalize(keys)
    scores = x_norm @ k_norm.T               [B, N]
    top_vals, top_idx = top_k(scores, 8)
    out = sum_k softmax(top_vals)_k * values[top_idx_k]

Two SPMD dispatches, no collectives.

Dispatch A (keys sharded along N: 12544 columns/core, all queries):
  - key prep: row inv-norms via ACT Square+accum, diag(1/||k||) built on
    GPSIMD, fp32 matmul transpose-and-scale -> kT bf16 [128, 2, 12544]
  - coarse scores in bf16 (fp32 PSUM accumulate), 512-wide matmuls
  - quad-slot reduction: 6 full groups of 2048 cols; within group,
    col P = 2048G + 512m + j belongs to slot S = 512G + j (member m).
    fold1 (DVE): max(pa, pb) pairs members (m, m+2); fold2 (GPSIMD):
    max of halves pairs member parity -> m2[S] = max over 4 members.
    Leftover cols 12288..12544 are single-member slots (ACT copy).
  - pack slot id (12 bits, linear) into the f32 mantissa, max8 -> per-core
    top-8 packed (value, slot), tie-free.
  -> output: packed candidates [B, 8] per core.

Host glue: normalize keys (numpy), build the quad-layout rescore tables:
  keysn4[c*NSLOT + S] = 4 normalized member key rows (pads zero),
  values4[4*(c*NSLOT+S) + m] = member value rows.

Dispatch B (queries sharded, 256 per core):
  - re-pack candidates by position, top-12 slots of 64 via
    max8 + match_replace + max8
  - slot -> quad row R = core*3328 + pid; ONE indirect gather per slot
    fetches all 4 member rows (4096B contiguous)
  - exact fp32 rescore of 48 members, top-8, softmax, value rows 4R+m
    gathered, weighted sum -> [256, 256] slice.
"""

import sys
import time

_TRN_REPO = "/opt/trn_rl_repo"
if _TRN_REPO not in sys.path:
    sys.path.insert(0, _TRN_REPO)

import numpy as np

import concourse.bass as bass
import concourse.mybir as mybir
import concourse.tile as tile
from concourse import bacc
from concourse.bass import IndirectOffsetOnAxis
from concourse.bass_utils import run_bass_kernel_spmd
from concourse.masks import make_identity

F32 = mybir.dt.float32
BF16 = mybir.dt.bfloat16
I32 = mybir.dt.int32
U32 = mybir.dt.uint32
ALU = mybir.AluOpType
ACTF = mybir.ActivationFunctionType
AX = mybir.AxisListType

# ---- problem constants ----
B = 2048
D = 256
N = 100000
TOPK = 8
NCORES = 8
NLOC = 12544              # key columns per core; 8*12544 = 100352 >= N
NSLOT = 3328              # 6*512 quad slots + 256 single-member leftover
NGRPF = 6                 # full quad groups of 2048 columns
NQUAD = NCORES * NSLOT    # rescore-table quad rows
NPADR = 4 * NQUAD         # member rows in values4
BSLOTS = 12               # slots rescored per query after the merge
NCAND = NCORES * 8

_PACK_MASK = 0xFFF
_NEG_BIG = -3.0e38


# --------------------------------------------------------------------------
# Dispatch A
# --------------------------------------------------------------------------

def build_dispatch_a(bq=B, dbg=False):
    qtiles = bq // 128
    kchunks = NLOC // 128          # 98

    nc = bacc.Bacc("TRN2", target_bir_lowering=False, debug=dbg)
    x_d = nc.dram_tensor("x", [bq, D], F32, kind="ExternalInput").ap()
    k_d = nc.dram_tensor("keys", [NLOC, D], F32, kind="ExternalInput").ap()
    out_d = nc.dram_tensor("cand", [bq, 8], F32, kind="ExternalOutput").ap()

    with tile.TileContext(nc) as tc:
        with (
            tc.tile_pool(name="const", bufs=1) as constp,
            tc.tile_pool(name="kprep", bufs=4) as kprep,
            tc.tile_pool(name="big", bufs=1) as bigp,
            tc.tile_pool(name="xp", bufs=2) as xp,
            tc.tile_pool(name="sp", bufs=4) as sp,
            tc.tile_pool(name="m1p", bufs=2) as m1p,
            tc.tile_pool(name="m2p", bufs=2) as m2p,
            tc.tile_pool(name="ps", bufs=2, space="PSUM") as psp,
        ):
            identb = constp.tile([128, 128], BF16)
            make_identity(nc, identb[:])
            identf = constp.tile([128, 128], F32)
            make_identity(nc, identf[:])
            eps = constp.tile([128, 1], F32)
            nc.gpsimd.memset(eps[:], 1e-30)
            iota_pack = constp.tile([128, NSLOT], I32)
            nc.gpsimd.iota(iota_pack[:], pattern=[[1, NSLOT]], base=0,
                           channel_multiplier=0)
            maskc = constp.tile([128, 1], I32)
            nc.gpsimd.memset(maskc[:], -(_PACK_MASK + 1))

            kT = bigp.tile([128, 2, NLOC], BF16)
            xT = bigp.tile([128, 2, bq], BF16)
            kinv_all = bigp.tile([128, kchunks], F32)

            # ---- x prep: normalize, cast, transpose into xT ----
            for qt in range(qtiles):
                xt = xp.tile([128, D], F32, tag="xt")
                nc.sync.dma_start(out=xt[:], in_=x_d[qt * 128:(qt + 1) * 128, :])
                xsq = xp.tile([128, D], F32, tag="xsq")
                xn2 = xp.tile([128, 1], F32, tag="xn2")
                nc.scalar.activation(xsq[:], xt[:], ACTF.Square, accum_out=xn2[:])
                xsrt = xp.tile([128, 1], F32, tag="xsrt")
                nc.scalar.activation(xsrt[:], xn2[:], ACTF.Sqrt)
                xinv = xp.tile([128, 1], F32, tag="xinv")
                nc.vector.reciprocal(xinv[:], xsrt[:])
                xnb = xp.tile([128, D], BF16, tag="xnb")
                nc.scalar.activation(xnb[:], xt[:], ACTF.Copy, scale=xinv[:])
                pt = psp.tile([128, 1024], F32, tag="pa")
                for c in range(2):
                    nc.tensor.matmul(pt[:, c * 128:(c + 1) * 128],
                                     lhsT=xnb[:, c * 128:(c + 1) * 128],
                                     rhs=identb[:], start=True, stop=True)
                nc.scalar.activation(
                    xT[:, :, qt * 128:(qt + 1) * 128], pt[:, :256], ACTF.Copy)

            # ---- key prep: 4-chunk groups share the sqrt ----
            for g4 in range(kchunks // 4 + 1):
                chunks = range(4 * g4, min(4 * g4 + 4, kchunks))
                if not chunks:
                    break
                nch = len(chunks)
                ktfs = []
                kn2g = kprep.tile([128, 4], F32, tag="kn2g")
                for i, ch in enumerate(chunks):
                    ktf = kprep.tile([128, D], F32, tag=f"ktf{i}")
                    nc.sync.dma_start(out=ktf[:],
                                      in_=k_d[ch * 128:(ch + 1) * 128, :])
                    ksq = kprep.tile([128, D], F32, tag=f"ksq{i % 2}")
                    nc.scalar.activation(ksq[:], ktf[:], ACTF.Square,
                                         accum_out=kn2g[:, i:i + 1])
                    ktfs.append(ktf)
                ksrt = kprep.tile([128, 4], F32, tag="ksrt")
                nc.scalar.activation(ksrt[:, :nch], kn2g[:, :nch], ACTF.Sqrt,
                                     bias=eps[:])
                kinvg = kprep.tile([128, 4], F32, tag="kinvg")
                nc.vector.reciprocal(kinvg[:, :nch], ksrt[:, :nch])
                nc.vector.tensor_copy(
                    kinv_all[:, 4 * g4:4 * g4 + nch], kinvg[:, :nch])
                for i, ch in enumerate(chunks):
                    diag = kprep.tile([128, 128], F32, tag=f"diag{i}")
                    nc.gpsimd.tensor_tensor(
                        diag[:], identf[:],
                        kinvg[:, i:i + 1].to_broadcast([128, 128]),
                        op=ALU.mult)
                    pt = psp.tile([128, 1024], F32, tag="pb")
                    for c in range(2):
                        nc.tensor.matmul(
                            pt[:, c * 128:(c + 1) * 128],
                            lhsT=ktfs[i][:, c * 128:(c + 1) * 128],
                            rhs=diag[:], start=True, stop=True)
                    nc.scalar.activation(
                        kT[:, :, ch * 128:(ch + 1) * 128], pt[:, :256],
                        ACTF.Copy)

            # ---- main loop ----
            for qt in range(qtiles):
                qs = slice(qt * 128, (qt + 1) * 128)
                m2 = m2p.tile([128, NSLOT], F32, tag="m2")
                for g in range(NGRPF):
                    base = 2048 * g
                    pa = psp.tile([128, 1024], F32, tag="pa")
                    pb = psp.tile([128, 1024], F32, tag="pb")
                    for half, pp in ((0, pa), (1, pa), (2, pb), (3, pb)):
                        dst = pp[:, (half % 2) * 512:(half % 2 + 1) * 512]
                        cs = base + half * 512
                        for c in range(2):
                            nc.tensor.matmul(
                                dst, lhsT=xT[:, c, qs],
                                rhs=kT[:, c, cs:cs + 512],
                                start=(c == 0), stop=(c == 1))
                    stmp = sp.tile([128, 1024], F32, tag="stmp")
                    nc.scalar.activation(stmp[:], pa[:], ACTF.Copy)
                    m1g = m1p.tile([128, 1024], F32, tag="m1g")
                    nc.vector.tensor_tensor(m1g[:], pb[:], stmp[:], op=ALU.max)
                    nc.vector.tensor_tensor(
                        m2[:, 512 * g:512 * (g + 1)],
                        m1g[:, :512], m1g[:, 512:], op=ALU.max)
                # leftover single-member slots
                pa = psp.tile([128, 1024], F32, tag="pa")
                for c in range(2):
                    nc.tensor.matmul(pa[:, :256], lhsT=xT[:, c, qs],
                                     rhs=kT[:, c, 12288:12544],
                                     start=(c == 0), stop=(c == 1))
                nc.scalar.activation(m2[:, 3072:3328], pa[:, :256], ACTF.Copy)

                nc.vector.scalar_tensor_tensor(
                    m2[:].bitcast(I32), m2[:].bitcast(I32),
                    maskc[:], iota_pack[:],
                    op0=ALU.bitwise_and, op1=ALU.bitwise_or)
                top = m2p.tile([128, 8], F32, tag="top")
                nc.vector.max(out=top[:], in_=m2[:])
                nc.sync.dma_start(out=out_d[qt * 128:(qt + 1) * 128, :],
                                  in_=top[:])

    nc.compile()
    return nc


# --------------------------------------------------------------------------
# Dispatch B
# --------------------------------------------------------------------------

def build_dispatch_b(bq_slice, dbg=False):
    qtiles = bq_slice // 128
    nmemb = BSLOTS * 4            # 48 rescored members

    nc = bacc.Bacc("TRN2", target_bir_lowering=False, debug=dbg)
    v_d = nc.dram_tensor("vals", [bq_slice, NCAND], F32,
                         kind="ExternalInput").ap()
    x_d = nc.dram_tensor("x", [bq_slice, D], F32, kind="ExternalInput").ap()
    k4_d = nc.dram_tensor("keysn4", [NQUAD, 4 * D], F32,
                          kind="ExternalInput").ap()
    val4_d = nc.dram_tensor("values4", [NPADR, D], F32,
                            kind="ExternalInput").ap()
    out_d = nc.dram_tensor("out", [bq_slice, D], F32, kind="ExternalOutput").ap()

    with tile.TileContext(nc) as tc:
        with (
            tc.tile_pool(name="const", bufs=1) as constp,
            tc.tile_pool(name="wp", bufs=2) as wp,
            tc.tile_pool(name="gp", bufs=2) as gp,
        ):
            iota_cand_i = constp.tile([128, NCAND], I32)
            nc.gpsimd.iota(iota_cand_i[:], pattern=[[1, NCAND]], base=0,
                           channel_multiplier=0)
            iota_cand_f = constp.tile([128, NCAND], F32)
            nc.gpsimd.tensor_copy(iota_cand_f[:], iota_cand_i[:])
            base_tab = constp.tile([128, NCAND], F32)   # NSLOT * (pos >> 3)
            nc.gpsimd.iota(base_tab[:], pattern=[[NSLOT, NCAND // 8], [0, 8]],
                           base=0, channel_multiplier=0,
                           allow_small_or_imprecise_dtypes=True)
            mask_tab = constp.tile([128, NCAND], I32)
            nc.gpsimd.memset(mask_tab[:], -(_PACK_MASK + 1))
            # member-major tables for the 48 rescored entries: idx = m*12+s
            m_tab = constp.tile([128, nmemb], F32)      # member id m
            nc.gpsimd.iota(m_tab[:], pattern=[[1, 4], [0, BSLOTS]], base=0,
                           channel_multiplier=0,
                           allow_small_or_imprecise_dtypes=True)
            iota_m_f = constp.tile([128, nmemb], F32)   # 0..47
            nc.gpsimd.iota(iota_m_f[:], pattern=[[1, nmemb]], base=0,
                           channel_multiplier=0,
                           allow_small_or_imprecise_dtypes=True)

            for qt in range(qtiles):
                r0, r1 = qt * 128, (qt + 1) * 128

                # --- x_norm (exact fp32) ---
                xt = wp.tile([128, D], F32, tag="xt")
                nc.sync.dma_start(out=xt[:], in_=x_d[r0:r1, :])
                xsq = wp.tile([128, D], F32, tag="xsq")
                xn2 = wp.tile([128, 1], F32, tag="xn2")
                nc.scalar.activation(xsq[:], xt[:], ACTF.Square, accum_out=xn2[:])
                xsrt = wp.tile([128, 1], F32, tag="xsrt")
                nc.scalar.activation(xsrt[:], xn2[:], ACTF.Sqrt)
                xinv = wp.tile([128, 1], F32, tag="xinv")
                nc.vector.reciprocal(xinv[:], xsrt[:])
                xn = wp.tile([128, D], F32, tag="xn")
                nc.scalar.activation(xn[:], xt[:], ACTF.Copy, scale=xinv[:])

                # --- candidate tables ---
                vin = wp.tile([128, NCAND], F32, tag="vin")
                nc.sync.dma_start(out=vin[:], in_=v_d[r0:r1, :])
                jlow_i = wp.tile([128, NCAND], I32, tag="jlowi")
                nc.vector.tensor_scalar(jlow_i[:], vin[:].bitcast(I32),
                                        _PACK_MASK, None, op0=ALU.bitwise_and)
                jlow_f = wp.tile([128, NCAND], F32, tag="jlowf")
                nc.vector.tensor_copy(jlow_f[:], jlow_i[:])
                vb = wp.tile([128, NCAND], F32, tag="vb")
                nc.vector.tensor_tensor(vb[:].bitcast(I32), vin[:].bitcast(I32),
                                        mask_tab[:], op=ALU.bitwise_and)
                vb2 = wp.tile([128, NCAND], F32, tag="vb2")
                nc.vector.tensor_tensor(vb2[:].bitcast(I32), vb[:].bitcast(I32),
                                        iota_cand_i[:], op=ALU.bitwise_or)

                # --- prune to top-BSLOTS slots ---
                t12 = wp.tile([128, 16], F32, tag="t12")
                nc.vector.max(out=t12[:, 0:8], in_=vb2[:])
                vrep = wp.tile([128, NCAND], F32, tag="vrep")
                nc.vector.match_replace(out=vrep[:], in_to_replace=t12[:, 0:8],
                                        in_values=vb2[:], imm_value=_NEG_BIG)
                nc.vector.max(out=t12[:, 8:16], in_=vrep[:])
                pos_i = wp.tile([128, BSLOTS], I32, tag="posi")
                nc.vector.tensor_scalar(pos_i[:], t12[:, :BSLOTS].bitcast(I32),
                                        NCAND - 1, None, op0=ALU.bitwise_and)
                pos_f = wp.tile([128, BSLOTS], F32, tag="posf")
                nc.vector.tensor_copy(pos_f[:], pos_i[:])

                # --- winner slots: pid and core-base via one-hot ---
                j_f = wp.tile([128, BSLOTS], F32, tag="jf")
                cb_f = wp.tile([128, BSLOTS], F32, tag="cbf")
                ohtmp = wp.tile([128, NCAND], F32, tag="ohtmp")
                ohmul = wp.tile([128, NCAND], F32, tag="ohmul")
                for w in range(BSLOTS):
                    nc.vector.tensor_tensor(
                        ohtmp[:], iota_cand_f[:],
                        pos_f[:, w:w + 1].to_broadcast([128, NCAND]),
                        op=ALU.is_equal)
                    nc.vector.tensor_tensor(ohmul[:], ohtmp[:], jlow_f[:],
                                            op=ALU.mult)
                    nc.vector.tensor_reduce(j_f[:, w:w + 1], ohmul[:],
                                            axis=AX.X, op=ALU.add)
                    nc.vector.tensor_tensor(ohmul[:], ohtmp[:], base_tab[:],
                                            op=ALU.mult)
                    nc.vector.tensor_reduce(cb_f[:, w:w + 1], ohmul[:],
                                            axis=AX.X, op=ALU.add)

                # --- quad row R = core*NSLOT + pid ---
                R_f = wp.tile([128, BSLOTS], F32, tag="rf")
                nc.vector.tensor_tensor(R_f[:], j_f[:], cb_f[:], op=ALU.add)
                R_i = wp.tile([128, BSLOTS], I32, tag="ri")
                nc.vector.tensor_copy(R_i[:], R_f[:])

                # value rows rowv[m*12+s] = 4*R_s + m
                rowv = wp.tile([128, 4, BSLOTS], F32, tag="rowv")
                nc.vector.tensor_scalar(
                    rowv[:], R_f[:].unsqueeze(1).to_broadcast(
                        [128, 4, BSLOTS]),
                    4.0, None, op0=ALU.mult)
                nc.vector.tensor_tensor(
                    rowv[:].rearrange("p m s -> p (m s)"),
                    rowv[:].rearrange("p m s -> p (m s)"), m_tab[:],
                    op=ALU.add)

                # --- gather quad rows + exact rescore ---
                g = gp.tile([128, BSLOTS, 4 * D], F32, tag="g")
                for s in range(BSLOTS):
                    nc.gpsimd.indirect_dma_start(
                        out=g[:, s, :], out_offset=None, in_=k4_d[:],
                        in_offset=IndirectOffsetOnAxis(
                            ap=R_i[:, s:s + 1], axis=0))
                sco = wp.tile([128, 4, BSLOTS], F32, tag="sco")
                gm = g[:].rearrange("p s (m d) -> p s m d", d=D)
                xb = xn[:].unsqueeze(1).unsqueeze(1).to_broadcast(
                    [128, BSLOTS, 1, D])
                for m in range(4):
                    prod = gp.tile([128, BSLOTS, D], F32, tag="prod")
                    nc.vector.tensor_tensor(
                        prod[:], gm[:, :, m, :],
                        xb[:, :, 0, :], op=ALU.mult)
                    nc.vector.tensor_reduce(sco[:, m, :], prod[:], axis=AX.X,
                                            op=ALU.add)

                # --- exact top-8 of the 48 members ---
                scof = sco[:].rearrange("p m s -> p (m s)")
                top8 = wp.tile([128, 8], F32, tag="top8")
                nc.vector.max(out=top8[:], in_=scof)
                pos8 = wp.tile([128, 8], U32, tag="pos8")
                nc.vector.max_index(pos8[:], top8[:], scof)
                pos8f = wp.tile([128, 8], F32, tag="pos8f")
                nc.vector.tensor_copy(pos8f[:], pos8[:])

                # --- softmax ---
                sh = wp.tile([128, 8], F32, tag="sh")
                nc.vector.tensor_tensor(sh[:], top8[:],
                                        top8[:, 0:1].to_broadcast([128, 8]),
                                        op=ALU.subtract)
                ex = wp.tile([128, 8], F32, tag="ex")
                nc.scalar.activation(ex[:], sh[:], ACTF.Exp)
                es = wp.tile([128, 1], F32, tag="es")
                nc.vector.tensor_reduce(es[:], ex[:], axis=AX.X, op=ALU.add)
                esr = wp.tile([128, 1], F32, tag="esr")
                nc.vector.reciprocal(esr[:], es[:])
                wgt = wp.tile([128, 8], F32, tag="wgt")
                nc.vector.tensor_tensor(wgt[:], ex[:],
                                        esr[:].to_broadcast([128, 8]),
                                        op=ALU.mult)

                # --- winner value rows via one-hot over member index ---
                winr = wp.tile([128, 8], F32, tag="winr")
                ohm = wp.tile([128, nmemb], F32, tag="ohm")
                rowvf = rowv[:].rearrange("p m s -> p (m s)")
                for w in range(8):
                    nc.vector.tensor_tensor(
                        ohm[:], iota_m_f[:],
                        pos8f[:, w:w + 1].to_broadcast([128, nmemb]),
                        op=ALU.is_equal)
                    nc.vector.tensor_tensor(ohm[:], ohm[:], rowvf,
                                            op=ALU.mult)
                    nc.vector.tensor_reduce(winr[:, w:w + 1], ohm[:], axis=AX.X,
                                            op=ALU.add)
                winr_i = wp.tile([128, 8], I32, tag="winri")
                nc.vector.tensor_copy(winr_i[:], winr[:])

                # --- gather value rows, weighted sum ---
                vg = gp.tile([128, 8, D], F32, tag="vg")
                for k in range(8):
                    nc.gpsimd.indirect_dma_start(
                        out=vg[:, k, :], out_offset=None, in_=val4_d[:],
                        in_offset=IndirectOffsetOnAxis(ap=winr_i[:, k:k + 1],
                                                       axis=0))
                vw = gp.tile([128, 8, D], F32, tag="vw")
                nc.vector.tensor_tensor(
                    vw[:], vg[:],
                    wgt[:].unsqueeze(2).to_broadcast([128, 8, D]), op=ALU.mult)
                ot = wp.tile([128, D], F32, tag="ot")
                nc.vector.tensor_reduce(ot[:], vw[:].rearrange("p k d -> p d k"),
                                        axis=AX.X, op=ALU.add)
                nc.sync.dma_start(out=out_d[r0:r1, :], in_=ot[:])

    nc.compile()
    return nc


# --------------------------------------------------------------------------
# Host orchestration
# --------------------------------------------------------------------------

_CACHE = {}


def _get_programs():
    if "A" not in _CACHE:
        _CACHE["A"] = build_dispatch_a()
    if "B" not in _CACHE:
        _CACHE["B"] = build_dispatch_b(B // NCORES)
    return _CACHE["A"], _CACHE["B"]


def _quad_cols():
    """P(S, m) per core; -1 for pad members."""
    P = -np.ones((NSLOT, 4), np.int64)
    S = np.arange(3072)
    G, j = S // 512, S % 512
    for m in range(4):
        P[S, m] = 2048 * G + 512 * m + j
    Sl = np.arange(3072, NSLOT)
    P[Sl, 0] = 12288 + (Sl - 3072)
    return P


_P4 = _quad_cols()                               # [NSLOT, 4]
_P4_VALID = _P4 >= 0
_P4G = (np.arange(NCORES)[:, None, None] * NLOC + _P4[None])  # [8, NSLOT, 4]


def kernel(x, keys, values, top_k):
    assert int(top_k) == TOPK
    x = np.ascontiguousarray(np.asarray(x, dtype=np.float32))
    keys = np.asarray(keys, dtype=np.float32)
    values = np.asarray(values, dtype=np.float32)
    assert x.shape == (B, D) and keys.shape == (N, D) and values.shape == (N, D)

    keys_sh = np.zeros((NCORES * NLOC, D), dtype=np.float32)
    keys_sh[:N] = keys
    values_sh = np.zeros((NCORES * NLOC, D), dtype=np.float32)
    values_sh[:N] = values

    nc_a, nc_b = _get_programs()
    core_ids = list(range(NCORES))

    # ---- dispatch A ----
    in_maps_a = [
        {"x": x,
         "keys": np.ascontiguousarray(keys_sh[c * NLOC:(c + 1) * NLOC])}
        for c in range(NCORES)
    ]
    t0 = time.perf_counter()
    res_a = run_bass_kernel_spmd(nc_a, in_maps_a, core_ids)
    t1 = time.perf_counter()
    cand = np.concatenate([res_a.results[c]["cand"] for c in range(NCORES)],
                          axis=1)  # [B, 64]

    # ---- host glue: normalized quad tables ----
    norms = np.sqrt(np.einsum("nd,nd->n", keys_sh, keys_sh))
    kn_sh = keys_sh / np.maximum(norms, 1e-12)[:, None]
    valid = np.broadcast_to(_P4_VALID[None], _P4G.shape)
    k4 = np.zeros((NCORES, NSLOT, 4, D), dtype=np.float32)
    k4[valid] = kn_sh[_P4G[valid]]
    k4 = np.ascontiguousarray(k4.reshape(NQUAD, 4 * D))
    v4 = np.zeros((NCORES, NSLOT, 4, D), dtype=np.float32)
    v4[valid] = values_sh[_P4G[valid]]
    v4 = np.ascontiguousarray(v4.reshape(NPADR, D))

    # ---- dispatch B ----
    bs = B // NCORES
    in_maps_b = [
        {
            "vals": np.ascontiguousarray(cand[c * bs:(c + 1) * bs]),
            "x": np.ascontiguousarray(x[c * bs:(c + 1) * bs]),
            "keysn4": k4,
            "values4": v4,
        }
        for c in range(NCORES)
    ]
    t2 = time.perf_counter()
    res_b = run_bass_kernel_spmd(nc_b, in_maps_b, core_ids)
    t3 = time.perf_counter()
    out = np.concatenate([res_b.results[c]["out"] for c in range(NCORES)],
                         axis=0)
    kernel.last_walltimes = (t1 - t0, t3 - t2)
    return out.astype(np.float32)


# revision 6
# speedup vs baseline: 1.1169x; 1.1169x over previous
"""Distributed cosine-similarity kNN retrieval (EpisodicSDM) on 8 Trainium2 cores.

Reference computation:
    x_norm = normalize(x); k_norm = normalize(keys)
    scores = x_norm @ k_norm.T               [B, N]
    top_vals, top_idx = top_k(scores, 8)
    out = sum_k softmax(top_vals)_k * values[top_idx_k]

Two SPMD dispatches, no collectives.

Dispatch A (keys sharded along N: 12544 columns/core, all queries):
  - key prep: row inv-norms (squares alternate ACT/DVE to balance engines),
    diag(1/||k||) built on GPSIMD, fp32 matmul transpose-and-scale ->
    kT bf16 [128, 2, 12544]
  - coarse scores in bf16 (fp32 PSUM accumulate), 512-wide matmuls
  - quad-slot reduction: 6 full groups of 2048 cols; within group,
    col P = 2048G + 512m + j belongs to slot S = 512G + j (member m).
    fold1 (DVE): max(pa, pb) pairs members (m, m+2); fold2 (DVE):
    max of m1 halves pairs member parity -> m2[S] = max over 4 members.
    Leftover cols 12288..12544 are single-member slots (ACT copy).
  - qtile 0's groups are interleaved with the key-prep blocks so the main
    pipeline starts as soon as kT block 0 exists.
  - pack slot id (12 bits, linear) into the f32 mantissa, max8 -> per-core
    top-8 packed (value, slot), tie-free.
  -> output: packed candidates [B, 8] per core.

Host glue: normalize keys (numpy), build the quad-layout rescore tables:
  keysn4[c*NSLOT + S] = 4 normalized member key rows (pads zero),
  values4[4*(c*NSLOT+S) + m] = member value rows.

Dispatch B (queries sharded, 256 per core):
  - re-pack candidates by position, top-12 slots of 64 via
    max8 + match_replace + max8
  - slot -> quad row R = NSLOT*(pos>>3) + pid; ONE indirect gather per slot
    fetches all 4 member rows (4096B contiguous)
  - exact fp32 rescore of 48 members via fused tensor_tensor_reduce,
    top-8, softmax, value rows 4R+m gathered, weighted sum.
"""

import sys
import time

_TRN_REPO = "/opt/trn_rl_repo"
if _TRN_REPO not in sys.path:
    sys.path.insert(0, _TRN_REPO)

import numpy as np

import concourse.bass as bass
import concourse.mybir as mybir
import concourse.tile as tile
from concourse import bacc
from concourse.bass import IndirectOffsetOnAxis
from concourse.bass_utils import run_bass_kernel_spmd
from concourse.masks import make_identity

F32 = mybir.dt.float32
BF16 = mybir.dt.bfloat16
I32 = mybir.dt.int32
U32 = mybir.dt.uint32
ALU = mybir.AluOpType
ACTF = mybir.ActivationFunctionType
AX = mybir.AxisListType

# ---- problem constants ----
B = 2048
D = 256
N = 100000
TOPK = 8
NCORES = 8
NLOC = 12544              # key columns per core; 8*12544 = 100352 >= N
NSLOT = 3328              # 6*512 quad slots + 256 single-member leftover
NGRPF = 6                 # full quad groups of 2048 columns
NQUAD = NCORES * NSLOT    # rescore-table quad rows
NPADR = 4 * NQUAD         # member rows in values4
BSLOTS = 12               # slots rescored per query after the merge
NCAND = NCORES * 8

_PACK_MASK = 0xFFF
_NEG_BIG = -3.0e38


# --------------------------------------------------------------------------
# Dispatch A
# --------------------------------------------------------------------------

def build_dispatch_a(bq=B, dbg=False):
    qtiles = bq // 128
    kchunks = NLOC // 128          # 98

    nc = bacc.Bacc("TRN2", target_bir_lowering=False, debug=dbg)
    x_d = nc.dram_tensor("x", [bq, D], F32, kind="ExternalInput").ap()
    k_d = nc.dram_tensor("keys", [NLOC, D], F32, kind="ExternalInput").ap()
    out_d = nc.dram_tensor("cand", [bq, 8], F32, kind="ExternalOutput").ap()

    with tile.TileContext(nc) as tc:
        with (
            tc.tile_pool(name="const", bufs=1) as constp,
            tc.tile_pool(name="kprep", bufs=4) as kprep,
            tc.tile_pool(name="big", bufs=1) as bigp,
            tc.tile_pool(name="xp", bufs=2) as xp,
            tc.tile_pool(name="sp", bufs=4) as sp,
            tc.tile_pool(name="m1p", bufs=2) as m1p,
            tc.tile_pool(name="m2p", bufs=2) as m2p,
            tc.tile_pool(name="ps", bufs=2, space="PSUM") as psp,
        ):
            identb = constp.tile([128, 128], BF16)
            make_identity(nc, identb[:])
            identf = constp.tile([128, 128], F32)
            make_identity(nc, identf[:])
            eps = constp.tile([128, 1], F32)
            nc.gpsimd.memset(eps[:], 1e-30)
            iota_pack = constp.tile([128, NSLOT], I32)
            nc.gpsimd.iota(iota_pack[:], pattern=[[1, NSLOT]], base=0,
                           channel_multiplier=0)
            maskc = constp.tile([128, 1], I32)
            nc.gpsimd.memset(maskc[:], -(_PACK_MASK + 1))

            kT = bigp.tile([128, 2, NLOC], BF16)
            xT = bigp.tile([128, 2, bq], BF16)

            # ---- x prep: normalize, cast, transpose into xT ----
            for qt in range(qtiles):
                xt = xp.tile([128, D], F32, tag="xt")
                nc.sync.dma_start(out=xt[:], in_=x_d[qt * 128:(qt + 1) * 128, :])
                xsq = xp.tile([128, D], F32, tag="xsq")
                nc.vector.tensor_tensor(xsq[:], xt[:], xt[:], op=ALU.mult)
                xn2 = xp.tile([128, 1], F32, tag="xn2")
                nc.vector.tensor_reduce(xn2[:], xsq[:], axis=AX.X, op=ALU.add)
                xsrt = xp.tile([128, 1], F32, tag="xsrt")
                nc.scalar.activation(xsrt[:], xn2[:], ACTF.Sqrt)
                xinv = xp.tile([128, 1], F32, tag="xinv")
                nc.vector.reciprocal(xinv[:], xsrt[:])
                xnb = xp.tile([128, D], BF16, tag="xnb")
                nc.scalar.activation(xnb[:], xt[:], ACTF.Copy, scale=xinv[:])
                pt = psp.tile([128, 1024], F32, tag="pa")
                for c in range(2):
                    nc.tensor.matmul(pt[:, c * 128:(c + 1) * 128],
                                     lhsT=xnb[:, c * 128:(c + 1) * 128],
                                     rhs=identb[:], start=True, stop=True)
                nc.scalar.activation(
                    xT[:, :, qt * 128:(qt + 1) * 128], pt[:, :256], ACTF.Copy)

            # ---- helpers ----
            def key_prep_chunks(chunks):
                """Prep a run of 128-key chunks (multiple of 4 or the tail)."""
                for g4s in range(0, len(chunks), 4):
                    grp = chunks[g4s:g4s + 4]
                    nch = len(grp)
                    ktfs = []
                    kn2g = kprep.tile([128, 4], F32, tag="kn2g")
                    for i, ch in enumerate(grp):
                        ktf = kprep.tile([128, D], F32, tag=f"ktf{i}")
                        nc.sync.dma_start(out=ktf[:],
                                          in_=k_d[ch * 128:(ch + 1) * 128, :])
                        if ch % 2 == 0:
                            ksq = kprep.tile([128, D], F32, tag=f"ksq{i % 2}")
                            nc.scalar.activation(ksq[:], ktf[:], ACTF.Square,
                                                 accum_out=kn2g[:, i:i + 1])
                        else:
                            ksq = kprep.tile([128, D], F32, tag=f"ksq{i % 2}")
                            nc.vector.tensor_tensor(ksq[:], ktf[:], ktf[:],
                                                    op=ALU.mult)
                            nc.vector.tensor_reduce(kn2g[:, i:i + 1], ksq[:],
                                                    axis=AX.X, op=ALU.add)
                        ktfs.append(ktf)
                    ksrt = kprep.tile([128, 4], F32, tag="ksrt")
                    nc.scalar.activation(ksrt[:, :nch], kn2g[:, :nch],
                                         ACTF.Sqrt, bias=eps[:])
                    kinvg = kprep.tile([128, 4], F32, tag="kinvg")
                    nc.vector.reciprocal(kinvg[:, :nch], ksrt[:, :nch])
                    for i, ch in enumerate(grp):
                        diag = kprep.tile([128, 128], F32, tag=f"diag{i}")
                        nc.gpsimd.tensor_tensor(
                            diag[:], identf[:],
                            kinvg[:, i:i + 1].to_broadcast([128, 128]),
                            op=ALU.mult)
                        pt = psp.tile([128, 1024], F32, tag="pb")
                        for c in range(2):
                            nc.tensor.matmul(
                                pt[:, c * 128:(c + 1) * 128],
                                lhsT=ktfs[i][:, c * 128:(c + 1) * 128],
                                rhs=diag[:], start=True, stop=True)
                        nc.scalar.activation(
                            kT[:, :, ch * 128:(ch + 1) * 128], pt[:, :256],
                            ACTF.Copy)

            def main_group(qt, g, m2):
                qs = slice(qt * 128, (qt + 1) * 128)
                base = 2048 * g
                pa = psp.tile([128, 1024], F32, tag="pa")
                pb = psp.tile([128, 1024], F32, tag="pb")
                for half, pp in ((0, pa), (1, pa), (2, pb), (3, pb)):
                    dst = pp[:, (half % 2) * 512:(half % 2 + 1) * 512]
                    cs = base + half * 512
                    for c in range(2):
                        nc.tensor.matmul(
                            dst, lhsT=xT[:, c, qs],
                            rhs=kT[:, c, cs:cs + 512],
                            start=(c == 0), stop=(c == 1))
                stmp = sp.tile([128, 1024], F32, tag="stmp")
                nc.scalar.activation(stmp[:], pa[:], ACTF.Copy)
                m1g = m1p.tile([128, 1024], F32, tag="m1g")
                nc.vector.tensor_tensor(m1g[:], pb[:], stmp[:], op=ALU.max)
                nc.vector.tensor_tensor(
                    m2[:, 512 * g:512 * (g + 1)],
                    m1g[:, :512], m1g[:, 512:], op=ALU.max)

            def main_leftover(qt, m2):
                qs = slice(qt * 128, (qt + 1) * 128)
                pa = psp.tile([128, 1024], F32, tag="pa")
                for c in range(2):
                    nc.tensor.matmul(pa[:, :256], lhsT=xT[:, c, qs],
                                     rhs=kT[:, c, 12288:12544],
                                     start=(c == 0), stop=(c == 1))
                nc.scalar.activation(m2[:, 3072:3328], pa[:, :256], ACTF.Copy)

            def main_tail(qt, m2):
                nc.vector.scalar_tensor_tensor(
                    m2[:].bitcast(I32), m2[:].bitcast(I32),
                    maskc[:], iota_pack[:],
                    op0=ALU.bitwise_and, op1=ALU.bitwise_or)
                top = m2p.tile([128, 8], F32, tag="top")
                nc.vector.max(out=top[:], in_=m2[:])
                nc.sync.dma_start(out=out_d[qt * 128:(qt + 1) * 128, :],
                                  in_=top[:])

            # ---- qtile 0 interleaved with key prep blocks ----
            m2_0 = m2p.tile([128, NSLOT], F32, tag="m2")
            for g in range(NGRPF):
                key_prep_chunks(list(range(16 * g, 16 * (g + 1))))
                main_group(0, g, m2_0)
            key_prep_chunks([96, 97])
            main_leftover(0, m2_0)
            main_tail(0, m2_0)

            # ---- remaining qtiles ----
            for qt in range(1, qtiles):
                m2 = m2p.tile([128, NSLOT], F32, tag="m2")
                for g in range(NGRPF):
                    main_group(qt, g, m2)
                main_leftover(qt, m2)
                main_tail(qt, m2)

    nc.compile()
    return nc


# --------------------------------------------------------------------------
# Dispatch B
# --------------------------------------------------------------------------

def build_dispatch_b(bq_slice, dbg=False):
    qtiles = bq_slice // 128
    nmemb = BSLOTS * 4            # 48 rescored members
    GCH = 6                       # slots per gather chunk tile

    nc = bacc.Bacc("TRN2", target_bir_lowering=False, debug=dbg)
    v_d = nc.dram_tensor("vals", [bq_slice, NCAND], F32,
                         kind="ExternalInput").ap()
    x_d = nc.dram_tensor("x", [bq_slice, D], F32, kind="ExternalInput").ap()
    k4_d = nc.dram_tensor("keysn4", [NQUAD, 4 * D], F32,
                          kind="ExternalInput").ap()
    val4_d = nc.dram_tensor("values4", [NPADR, D], F32,
                            kind="ExternalInput").ap()
    out_d = nc.dram_tensor("out", [bq_slice, D], F32, kind="ExternalOutput").ap()

    with tile.TileContext(nc) as tc:
        with (
            tc.tile_pool(name="const", bufs=1) as constp,
            tc.tile_pool(name="wp", bufs=2) as wp,
            tc.tile_pool(name="gp", bufs=2) as gp,
        ):
            iota_cand_i = constp.tile([128, NCAND], I32)
            nc.gpsimd.iota(iota_cand_i[:], pattern=[[1, NCAND]], base=0,
                           channel_multiplier=0)
            iota_cand_f = constp.tile([128, NCAND], F32)
            nc.gpsimd.tensor_copy(iota_cand_f[:], iota_cand_i[:])
            mask_tab = constp.tile([128, NCAND], I32)
            nc.gpsimd.memset(mask_tab[:], -(_PACK_MASK + 1))
            # member-major tables for the 48 rescored entries: idx = m*12+s
            m_tab = constp.tile([128, nmemb], F32)      # member id m
            nc.gpsimd.iota(m_tab[:], pattern=[[1, 4], [0, BSLOTS]], base=0,
                           channel_multiplier=0,
                           allow_small_or_imprecise_dtypes=True)
            iota_m_f = constp.tile([128, nmemb], F32)   # 0..47
            nc.gpsimd.iota(iota_m_f[:], pattern=[[1, nmemb]], base=0,
                           channel_multiplier=0,
                           allow_small_or_imprecise_dtypes=True)

            for qt in range(qtiles):
                r0, r1 = qt * 128, (qt + 1) * 128

                # --- x_norm (exact fp32) ---
                xt = wp.tile([128, D], F32, tag="xt")
                nc.sync.dma_start(out=xt[:], in_=x_d[r0:r1, :])
                xsq = wp.tile([128, D], F32, tag="xsq")
                xn2 = wp.tile([128, 1], F32, tag="xn2")
                nc.scalar.activation(xsq[:], xt[:], ACTF.Square, accum_out=xn2[:])
                xsrt = wp.tile([128, 1], F32, tag="xsrt")
                nc.scalar.activation(xsrt[:], xn2[:], ACTF.Sqrt)
                xinv = wp.tile([128, 1], F32, tag="xinv")
                nc.vector.reciprocal(xinv[:], xsrt[:])
                xn = wp.tile([128, D], F32, tag="xn")
                nc.scalar.activation(xn[:], xt[:], ACTF.Copy, scale=xinv[:])

                # --- candidate tables ---
                vin = wp.tile([128, NCAND], F32, tag="vin")
                nc.sync.dma_start(out=vin[:], in_=v_d[r0:r1, :])
                jlow_i = wp.tile([128, NCAND], I32, tag="jlowi")
                nc.vector.tensor_scalar(jlow_i[:], vin[:].bitcast(I32),
                                        _PACK_MASK, None, op0=ALU.bitwise_and)
                jlow_f = wp.tile([128, NCAND], F32, tag="jlowf")
                nc.vector.tensor_copy(jlow_f[:], jlow_i[:])
                vb = wp.tile([128, NCAND], F32, tag="vb")
                nc.vector.tensor_tensor(vb[:].bitcast(I32), vin[:].bitcast(I32),
                                        mask_tab[:], op=ALU.bitwise_and)
                vb2 = wp.tile([128, NCAND], F32, tag="vb2")
                nc.vector.tensor_tensor(vb2[:].bitcast(I32), vb[:].bitcast(I32),
                                        iota_cand_i[:], op=ALU.bitwise_or)

                # --- prune to top-BSLOTS slots ---
                t12 = wp.tile([128, 16], F32, tag="t12")
                nc.vector.max(out=t12[:, 0:8], in_=vb2[:])
                vrep = wp.tile([128, NCAND], F32, tag="vrep")
                nc.vector.match_replace(out=vrep[:], in_to_replace=t12[:, 0:8],
                                        in_values=vb2[:], imm_value=_NEG_BIG)
                nc.vector.max(out=t12[:, 8:16], in_=vrep[:])
                pos_i = wp.tile([128, BSLOTS], I32, tag="posi")
                nc.vector.tensor_scalar(pos_i[:], t12[:, :BSLOTS].bitcast(I32),
                                        NCAND - 1, None, op0=ALU.bitwise_and)
                pos_f = wp.tile([128, BSLOTS], F32, tag="posf")
                nc.vector.tensor_copy(pos_f[:], pos_i[:])

                # --- winner slot pid via one-hot; core-base arithmetically ---
                j_f = wp.tile([128, BSLOTS], F32, tag="jf")
                ohmul = wp.tile([128, NCAND], F32, tag="ohmul")
                for w in range(BSLOTS):
                    nc.vector.scalar_tensor_tensor(
                        ohmul[:], iota_cand_f[:], pos_f[:, w:w + 1],
                        jlow_f[:], op0=ALU.is_equal, op1=ALU.mult)
                    nc.vector.tensor_reduce(j_f[:, w:w + 1], ohmul[:],
                                            axis=AX.X, op=ALU.add)
                cb_i = wp.tile([128, BSLOTS], I32, tag="cbi")
                nc.vector.tensor_scalar(cb_i[:], pos_i[:], ~7, None,
                                        op0=ALU.bitwise_and)   # 8*core
                cb_f = wp.tile([128, BSLOTS], F32, tag="cbf")
                nc.vector.tensor_copy(cb_f[:], cb_i[:])

                # --- quad row R = NSLOT*core + pid ---
                R_f = wp.tile([128, BSLOTS], F32, tag="rf")
                nc.vector.tensor_scalar(R_f[:], cb_f[:], float(NSLOT // 8),
                                        None, op0=ALU.mult)
                nc.vector.tensor_tensor(R_f[:], R_f[:], j_f[:], op=ALU.add)
                R_i = wp.tile([128, BSLOTS], I32, tag="ri")
                nc.vector.tensor_copy(R_i[:], R_f[:])

                # value rows rowv[m*12+s] = 4*R_s + m
                rowv = wp.tile([128, 4, BSLOTS], F32, tag="rowv")
                nc.vector.tensor_scalar(
                    rowv[:], R_f[:].unsqueeze(1).to_broadcast(
                        [128, 4, BSLOTS]),
                    4.0, None, op0=ALU.mult)
                nc.vector.tensor_tensor(
                    rowv[:].rearrange("p m s -> p (m s)"),
                    rowv[:].rearrange("p m s -> p (m s)"), m_tab[:],
                    op=ALU.add)

                # --- gather quad rows (chunked) + exact rescore ---
                sco = wp.tile([128, 4, BSLOTS], F32, tag="sco")
                xb = xn[:].unsqueeze(1).to_broadcast([128, GCH, D])
                for s0 in range(0, BSLOTS, GCH):
                    g = gp.tile([128, GCH, 4 * D], F32, tag=f"g{s0 // GCH}")
                    for s in range(GCH):
                        nc.gpsimd.indirect_dma_start(
                            out=g[:, s, :], out_offset=None, in_=k4_d[:],
                            in_offset=IndirectOffsetOnAxis(
                                ap=R_i[:, s0 + s:s0 + s + 1], axis=0))
                    gm = g[:].rearrange("p s (m d) -> p s m d", d=D)
                    for m in range(4):
                        prod = gp.tile([128, GCH, D], F32, tag="prod")
                        nc.vector.tensor_tensor(
                            prod[:], gm[:, :, m, :], xb, op=ALU.mult)
                        nc.vector.tensor_reduce(
                            sco[:, m, s0:s0 + GCH], prod[:], axis=AX.X,
                            op=ALU.add)

                # --- exact top-8 of the 48 members ---
                scof = sco[:].rearrange("p m s -> p (m s)")
                top8 = wp.tile([128, 8], F32, tag="top8")
                nc.vector.max(out=top8[:], in_=scof)
                pos8 = wp.tile([128, 8], U32, tag="pos8")
                nc.vector.max_index(pos8[:], top8[:], scof)
                pos8f = wp.tile([128, 8], F32, tag="pos8f")
                nc.vector.tensor_copy(pos8f[:], pos8[:])

                # --- softmax ---
                sh = wp.tile([128, 8], F32, tag="sh")
                nc.vector.tensor_tensor(sh[:], top8[:],
                                        top8[:, 0:1].to_broadcast([128, 8]),
                                        op=ALU.subtract)
                ex = wp.tile([128, 8], F32, tag="ex")
                nc.scalar.activation(ex[:], sh[:], ACTF.Exp)
                es = wp.tile([128, 1], F32, tag="es")
                nc.vector.tensor_reduce(es[:], ex[:], axis=AX.X, op=ALU.add)
                esr = wp.tile([128, 1], F32, tag="esr")
                nc.vector.reciprocal(esr[:], es[:])
                wgt = wp.tile([128, 8], F32, tag="wgt")
                nc.vector.tensor_tensor(wgt[:], ex[:],
                                        esr[:].to_broadcast([128, 8]),
                                        op=ALU.mult)

                # --- winner value rows via one-hot over member index ---
                winr = wp.tile([128, 8], F32, tag="winr")
                ohm = wp.tile([128, nmemb], F32, tag="ohm")
                rowvf = rowv[:].rearrange("p m s -> p (m s)")
                for w in range(8):
                    nc.vector.scalar_tensor_tensor(
                        ohm[:], iota_m_f[:], pos8f[:, w:w + 1], rowvf,
                        op0=ALU.is_equal, op1=ALU.mult)
                    nc.vector.tensor_reduce(winr[:, w:w + 1], ohm[:], axis=AX.X,
                                            op=ALU.add)
                winr_i = wp.tile([128, 8], I32, tag="winri")
                nc.vector.tensor_copy(winr_i[:], winr[:])

                # --- gather value rows, weighted sum ---
                vg = gp.tile([128, 8, D], F32, tag="vg")
                for k in range(8):
                    nc.gpsimd.indirect_dma_start(
                        out=vg[:, k, :], out_offset=None, in_=val4_d[:],
                        in_offset=IndirectOffsetOnAxis(ap=winr_i[:, k:k + 1],
                                                       axis=0))
                vw = gp.tile([128, 8, D], F32, tag="vw")
                nc.vector.tensor_tensor(
                    vw[:], vg[:],
                    wgt[:].unsqueeze(2).to_broadcast([128, 8, D]), op=ALU.mult)
                ot = wp.tile([128, D], F32, tag="ot")
                nc.vector.tensor_reduce(ot[:], vw[:].rearrange("p k d -> p d k"),
                                        axis=AX.X, op=ALU.add)
                nc.sync.dma_start(out=out_d[r0:r1, :], in_=ot[:])

    nc.compile()
    return nc


# --------------------------------------------------------------------------
# Host orchestration
# --------------------------------------------------------------------------

_CACHE = {}


def _get_programs():
    if "A" not in _CACHE:
        _CACHE["A"] = build_dispatch_a()
    if "B" not in _CACHE:
        _CACHE["B"] = build_dispatch_b(B // NCORES)
    return _CACHE["A"], _CACHE["B"]


def _quad_cols():
    """P(S, m) per core; -1 for pad members."""
    P = -np.ones((NSLOT, 4), np.int64)
    S = np.arange(3072)
    G, j = S // 512, S % 512
    for m in range(4):
        P[S, m] = 2048 * G + 512 * m + j
    Sl = np.arange(3072, NSLOT)
    P[Sl, 0] = 12288 + (Sl - 3072)
    return P


_P4 = _quad_cols()                               # [NSLOT, 4]
_P4_VALID = _P4 >= 0
_P4G = (np.arange(NCORES)[:, None, None] * NLOC + _P4[None])  # [8, NSLOT, 4]


def kernel(x, keys, values, top_k):
    assert int(top_k) == TOPK
    x = np.ascontiguousarray(np.asarray(x, dtype=np.float32))
    keys = np.asarray(keys, dtype=np.float32)
    values = np.asarray(values, dtype=np.float32)
    assert x.shape == (B, D) and keys.shape == (N, D) and values.shape == (N, D)

    keys_sh = np.zeros((NCORES * NLOC, D), dtype=np.float32)
    keys_sh[:N] = keys
    values_sh = np.zeros((NCORES * NLOC, D), dtype=np.float32)
    values_sh[:N] = values

    nc_a, nc_b = _get_programs()
    core_ids = list(range(NCORES))

    # ---- dispatch A ----
    in_maps_a = [
        {"x": x,
         "keys": np.ascontiguousarray(keys_sh[c * NLOC:(c + 1) * NLOC])}
        for c in range(NCORES)
    ]
    t0 = time.perf_counter()
    res_a = run_bass_kernel_spmd(nc_a, in_maps_a, core_ids)
    t1 = time.perf_counter()
    cand = np.concatenate([res_a.results[c]["cand"] for c in range(NCORES)],
                          axis=1)  # [B, 64]

    # ---- host glue: normalized quad tables ----
    norms = np.sqrt(np.einsum("nd,nd->n", keys_sh, keys_sh))
    kn_sh = keys_sh / np.maximum(norms, 1e-12)[:, None]
    valid = np.broadcast_to(_P4_VALID[None], _P4G.shape)
    k4 = np.zeros((NCORES, NSLOT, 4, D), dtype=np.float32)
    k4[valid] = kn_sh[_P4G[valid]]
    k4 = np.ascontiguousarray(k4.reshape(NQUAD, 4 * D))
    v4 = np.zeros((NCORES, NSLOT, 4, D), dtype=np.float32)
    v4[valid] = values_sh[_P4G[valid]]
    v4 = np.ascontiguousarray(v4.reshape(NPADR, D))

    # ---- dispatch B ----
    bs = B // NCORES
    in_maps_b = [
        {
            "vals": np.ascontiguousarray(cand[c * bs:(c + 1) * bs]),
            "x": np.ascontiguousarray(x[c * bs:(c + 1) * bs]),
            "keysn4": k4,
            "values4": v4,
        }
        for c in range(NCORES)
    ]
    t2 = time.perf_counter()
    res_b = run_bass_kernel_spmd(nc_b, in_maps_b, core_ids)
    t3 = time.perf_counter()
    out = np.concatenate([res_b.results[c]["out"] for c in range(NCORES)],
                         axis=0)
    kernel.last_walltimes = (t1 - t0, t3 - t2)
    return out.astype(np.float32)


# revision 15
# speedup vs baseline: 1.2252x; 1.0970x over previous
"""Distributed cosine-similarity kNN retrieval (EpisodicSDM) on 8 Trainium2 cores.

Reference computation:
    x_norm = normalize(x); k_norm = normalize(keys)
    scores = x_norm @ k_norm.T               [B, N]
    top_vals, top_idx = top_k(scores, 8)
    out = sum_k softmax(top_vals)_k * values[top_idx_k]

Two SPMD dispatches, no collectives.

Dispatch A (keys sharded along N: 12544 columns/core, all queries):
  - key prep: row inv-norms (squares alternate ACT/DVE to balance engines),
    diag(1/||k||) built on GPSIMD, fp32 matmul transpose-and-scale ->
    kT bf16 [128, 2, 12544]
  - coarse scores in bf16 (fp32 PSUM accumulate), 512-wide matmuls
  - quad-slot reduction: 6 full groups of 2048 cols; within group,
    col P = 2048G + 512m + j belongs to slot S = 512G + j (member m).
    fold1 (DVE): max(pa, pb) pairs members (m, m+2); fold2 (DVE):
    max of m1 halves pairs member parity -> m2[S] = max over 4 members.
    Leftover cols 12288..12544 are single-member slots (ACT copy).
  - qtile 0's groups are interleaved with the key-prep blocks so the main
    pipeline starts as soon as kT block 0 exists.
  - pack slot id (12 bits, linear) into the f32 mantissa, max8 -> per-core
    top-8 packed (value, slot), tie-free.
  -> output: packed candidates [B, 8] per core.

Host glue: normalize keys (numpy), build the quad-layout rescore tables:
  keysn4[c*NSLOT + S] = 4 normalized member key rows (pads zero),
  values4[4*(c*NSLOT+S) + m] = member value rows.

Dispatch B (queries sharded, 256 per core):
  - re-pack candidates by position, top-12 slots of 64 via
    max8 + match_replace + max8
  - slot -> quad row R = NSLOT*(pos>>3) + pid; ONE indirect gather per slot
    fetches all 4 member rows (4096B contiguous)
  - exact fp32 rescore of 48 members via fused tensor_tensor_reduce,
    top-8, softmax, value rows 4R+m gathered, weighted sum.
"""

import sys
import time

_TRN_REPO = "/opt/trn_rl_repo"
if _TRN_REPO not in sys.path:
    sys.path.insert(0, _TRN_REPO)

import numpy as np

import concourse.bass as bass
import concourse.mybir as mybir
import concourse.tile as tile
from concourse import bacc
from concourse.bass import IndirectOffsetOnAxis
from concourse.bass_utils import run_bass_kernel_spmd
from concourse.masks import make_identity

F32 = mybir.dt.float32
BF16 = mybir.dt.bfloat16
I32 = mybir.dt.int32
U32 = mybir.dt.uint32
ALU = mybir.AluOpType
ACTF = mybir.ActivationFunctionType
AX = mybir.AxisListType

# ---- problem constants ----
B = 2048
D = 256
N = 100000
TOPK = 8
NCORES = 8
NLOC = 12544              # key columns per core; 8*12544 = 100352 >= N
NSLOT = 3328              # 6*512 quad slots + 256 single-member leftover
NGRPF = 6                 # full quad groups of 2048 columns
NQUAD = NCORES * NSLOT    # rescore-table quad rows
NPADR = 4 * NQUAD         # member rows in values4
BSLOTS = 12               # slots rescored per query after the merge
NCAND = NCORES * 8

_PACK_MASK = 0xFFF
_NEG_BIG = -3.0e38


# --------------------------------------------------------------------------
# Dispatch A
# --------------------------------------------------------------------------

def build_dispatch_a(bq=B, dbg=False):
    qtiles = bq // 128
    kchunks = NLOC // 128          # 98

    nc = bacc.Bacc("TRN2", target_bir_lowering=False, debug=dbg)
    x_d = nc.dram_tensor("x", [bq, D], F32, kind="ExternalInput").ap()
    k_d = nc.dram_tensor("keys", [NLOC, D], F32, kind="ExternalInput").ap()
    out_d = nc.dram_tensor("cand", [bq, 8], F32, kind="ExternalOutput").ap()

    with tile.TileContext(nc) as tc:
        with (
            tc.tile_pool(name="const", bufs=1) as constp,
            tc.tile_pool(name="kprep", bufs=4) as kprep,
            tc.tile_pool(name="big", bufs=1) as bigp,
            tc.tile_pool(name="xp", bufs=2) as xp,
            tc.tile_pool(name="sp", bufs=4) as sp,
            tc.tile_pool(name="m1p", bufs=2) as m1p,
            tc.tile_pool(name="m2p", bufs=4) as m2p,
            tc.tile_pool(name="ps", bufs=2, space="PSUM") as psp,
        ):
            identb = constp.tile([128, 128], BF16)
            make_identity(nc, identb[:])
            identf = constp.tile([128, 128], F32)
            make_identity(nc, identf[:])
            eps = constp.tile([128, 1], F32)
            nc.gpsimd.memset(eps[:], 1e-30)
            iota_pack = constp.tile([128, NSLOT], I32)
            nc.gpsimd.iota(iota_pack[:], pattern=[[1, NSLOT]], base=0,
                           channel_multiplier=0)
            maskc = constp.tile([128, 1], I32)
            nc.gpsimd.memset(maskc[:], -(_PACK_MASK + 1))

            kT = bigp.tile([128, 2, NLOC], BF16)
            xT = bigp.tile([128, 2, bq], BF16)

            # ---- x prep: normalize, cast, transpose into xT ----
            for qt in range(qtiles):
                xt = xp.tile([128, D], F32, tag="xt")
                nc.sync.dma_start(out=xt[:], in_=x_d[qt * 128:(qt + 1) * 128, :])
                xsq = xp.tile([128, D], F32, tag="xsq")
                nc.vector.tensor_tensor(xsq[:], xt[:], xt[:], op=ALU.mult)
                xn2 = xp.tile([128, 1], F32, tag="xn2")
                nc.vector.tensor_reduce(xn2[:], xsq[:], axis=AX.X, op=ALU.add)
                xsrt = xp.tile([128, 1], F32, tag="xsrt")
                nc.scalar.activation(xsrt[:], xn2[:], ACTF.Sqrt)
                xinv = xp.tile([128, 1], F32, tag="xinv")
                nc.vector.reciprocal(xinv[:], xsrt[:])
                xnb = xp.tile([128, D], BF16, tag="xnb")
                nc.scalar.activation(xnb[:], xt[:], ACTF.Copy, scale=xinv[:])
                pt = psp.tile([128, 1024], F32, tag="pa")
                for c in range(2):
                    nc.tensor.matmul(pt[:, c * 128:(c + 1) * 128],
                                     lhsT=xnb[:, c * 128:(c + 1) * 128],
                                     rhs=identb[:], start=True, stop=True)
                nc.scalar.activation(
                    xT[:, :, qt * 128:(qt + 1) * 128], pt[:, :256], ACTF.Copy)

            # ---- helpers ----
            def key_prep_chunks(chunks):
                """Prep a run of 128-key chunks (multiple of 4 or the tail)."""
                for g4s in range(0, len(chunks), 4):
                    grp = chunks[g4s:g4s + 4]
                    nch = len(grp)
                    ktfs = []
                    kn2g = kprep.tile([128, 4], F32, tag="kn2g")
                    for i, ch in enumerate(grp):
                        ktf = kprep.tile([128, D], F32, tag=f"ktf{i}")
                        nc.sync.dma_start(out=ktf[:],
                                          in_=k_d[ch * 128:(ch + 1) * 128, :])
                        if ch % 4 == 0:
                            ksq = kprep.tile([128, D], F32, tag=f"ksq{i % 2}")
                            nc.scalar.activation(ksq[:], ktf[:], ACTF.Square,
                                                 accum_out=kn2g[:, i:i + 1])
                        else:
                            ksq = kprep.tile([128, D], F32, tag=f"ksq{i % 2}")
                            nc.vector.tensor_tensor(ksq[:], ktf[:], ktf[:],
                                                    op=ALU.mult)
                            nc.vector.tensor_reduce(kn2g[:, i:i + 1], ksq[:],
                                                    axis=AX.X, op=ALU.add)
                        ktfs.append(ktf)
                    ksrt = kprep.tile([128, 4], F32, tag="ksrt")
                    nc.scalar.activation(ksrt[:, :nch], kn2g[:, :nch],
                                         ACTF.Sqrt, bias=eps[:])
                    kinvg = kprep.tile([128, 4], F32, tag="kinvg")
                    nc.vector.reciprocal(kinvg[:, :nch], ksrt[:, :nch])
                    for i, ch in enumerate(grp):
                        diag = kprep.tile([128, 128], F32, tag=f"diag{i}")
                        nc.gpsimd.tensor_tensor(
                            diag[:], identf[:],
                            kinvg[:, i:i + 1].to_broadcast([128, 128]),
                            op=ALU.mult)
                        pt = psp.tile([128, 1024], F32, tag="pb")
                        for c in range(2):
                            nc.tensor.matmul(
                                pt[:, c * 128:(c + 1) * 128],
                                lhsT=ktfs[i][:, c * 128:(c + 1) * 128],
                                rhs=diag[:], start=True, stop=True)
                        nc.scalar.activation(
                            kT[:, :, ch * 128:(ch + 1) * 128], pt[:, :256],
                            ACTF.Copy)

            NBF = 4   # groups using bf16 folds (ACT copies both PSUM banks)

            def main_group(qt, g, m2):
                qs = slice(qt * 128, (qt + 1) * 128)
                base = 2048 * g
                pa = psp.tile([128, 1024], F32, tag="pa")
                pb = psp.tile([128, 1024], F32, tag="pb")
                for half, pp in ((0, pa), (1, pa), (2, pb), (3, pb)):
                    dst = pp[:, (half % 2) * 512:(half % 2 + 1) * 512]
                    cs = base + half * 512
                    for c in range(2):
                        nc.tensor.matmul(
                            dst, lhsT=xT[:, c, qs],
                            rhs=kT[:, c, cs:cs + 512],
                            start=(c == 0), stop=(c == 1))
                if g < NBF:
                    # bf16 path: ACT casts both banks, DVE folds at 2x
                    sa = sp.tile([128, 1024], BF16, tag="sa")
                    nc.scalar.activation(sa[:], pa[:], ACTF.Copy)
                    sb = sp.tile([128, 1024], BF16, tag="sb")
                    nc.scalar.activation(sb[:], pb[:], ACTF.Copy)
                    m1b = m1p.tile([128, 1024], BF16, tag="m1b")
                    nc.vector.tensor_tensor(m1b[:], sa[:], sb[:], op=ALU.max)
                    nc.vector.tensor_tensor(
                        m2[:, 512 * g:512 * (g + 1)],
                        m1b[:, :512], m1b[:, 512:], op=ALU.max)
                else:
                    stmp = sp.tile([128, 1024], F32, tag="stmp")
                    nc.scalar.activation(stmp[:], pa[:], ACTF.Copy)
                    m1g = m1p.tile([128, 1024], F32, tag="m1g")
                    nc.vector.tensor_tensor(m1g[:], pb[:], stmp[:], op=ALU.max)
                    nc.vector.tensor_tensor(
                        m2[:, 512 * g:512 * (g + 1)],
                        m1g[:, :512], m1g[:, 512:], op=ALU.max)

            def main_leftover(qt, m2):
                qs = slice(qt * 128, (qt + 1) * 128)
                pa = psp.tile([128, 1024], F32, tag="pa")
                for c in range(2):
                    nc.tensor.matmul(pa[:, :256], lhsT=xT[:, c, qs],
                                     rhs=kT[:, c, 12288:12544],
                                     start=(c == 0), stop=(c == 1))
                nc.scalar.activation(m2[:, 3072:3328], pa[:, :256], ACTF.Copy)

            def main_tail(qt, m2):
                nc.vector.scalar_tensor_tensor(
                    m2[:].bitcast(I32), m2[:].bitcast(I32),
                    maskc[:], iota_pack[:],
                    op0=ALU.bitwise_and, op1=ALU.bitwise_or)
                top = m2p.tile([128, 8], F32, tag="top")
                nc.vector.max(out=top[:], in_=m2[:])
                nc.sync.dma_start(out=out_d[qt * 128:(qt + 1) * 128, :],
                                  in_=top[:])

            # ---- qtiles 0..3 interleaved with key prep blocks ----
            NIL = 4
            m2s = []
            for _i in range(NIL):
                m2q = m2p.tile([128, NSLOT], F32, tag="m2")
                m2s.append(m2q)
            key_prep_chunks(list(range(0, 16)))
            for g in range(NGRPF):
                if g + 1 < NGRPF:
                    # overlap mains of ready block g with prep of block g+1
                    for q in range(NIL):
                        key_prep_chunks(list(range(16 * (g + 1) + 4 * q,
                                                   16 * (g + 1) + 4 * (q + 1))))
                        main_group(q, g, m2s[q])
                else:
                    key_prep_chunks([96, 97])
                    for q in range(NIL):
                        main_group(q, g, m2s[q])
            for q in range(NIL):
                main_leftover(q, m2s[q])
                main_tail(q, m2s[q])

            # ---- remaining qtiles ----
            for qt in range(NIL, qtiles):
                m2 = m2p.tile([128, NSLOT], F32, tag="m2")
                for g in range(NGRPF):
                    main_group(qt, g, m2)
                main_leftover(qt, m2)
                main_tail(qt, m2)

    nc.compile()
    return nc


# --------------------------------------------------------------------------
# Dispatch B
# --------------------------------------------------------------------------

def build_dispatch_b(bq_slice, dbg=False):
    qtiles = bq_slice // 128
    nmemb = BSLOTS * 4            # 48 rescored members
    GCH = 6                       # slots per gather chunk tile

    nc = bacc.Bacc("TRN2", target_bir_lowering=False, debug=dbg)
    v_d = nc.dram_tensor("vals", [bq_slice, NCAND], F32,
                         kind="ExternalInput").ap()
    x_d = nc.dram_tensor("x", [bq_slice, D], F32, kind="ExternalInput").ap()
    k4_d = nc.dram_tensor("keysn4", [NQUAD, 4 * D], F32,
                          kind="ExternalInput").ap()
    val4_d = nc.dram_tensor("values4", [NPADR, D], F32,
                            kind="ExternalInput").ap()
    out_d = nc.dram_tensor("out", [bq_slice, D], F32, kind="ExternalOutput").ap()

    with tile.TileContext(nc) as tc:
        with (
            tc.tile_pool(name="const", bufs=1) as constp,
            tc.tile_pool(name="wp", bufs=2) as wp,
            tc.tile_pool(name="gp", bufs=2) as gp,
        ):
            iota_cand_i = constp.tile([128, NCAND], I32)
            nc.gpsimd.iota(iota_cand_i[:], pattern=[[1, NCAND]], base=0,
                           channel_multiplier=0)
            iota_cand_f = constp.tile([128, NCAND], F32)
            nc.gpsimd.tensor_copy(iota_cand_f[:], iota_cand_i[:])
            mask_tab = constp.tile([128, NCAND], I32)
            nc.gpsimd.memset(mask_tab[:], -(_PACK_MASK + 1))
            # member-major tables for the 48 rescored entries: idx = m*12+s
            m_tab = constp.tile([128, nmemb], F32)      # member id m
            nc.gpsimd.iota(m_tab[:], pattern=[[1, 4], [0, BSLOTS]], base=0,
                           channel_multiplier=0,
                           allow_small_or_imprecise_dtypes=True)
            iota_m_f = constp.tile([128, nmemb], F32)   # 0..47
            nc.gpsimd.iota(iota_m_f[:], pattern=[[1, nmemb]], base=0,
                           channel_multiplier=0,
                           allow_small_or_imprecise_dtypes=True)

            for qt in range(qtiles):
                r0, r1 = qt * 128, (qt + 1) * 128

                # --- x_norm (exact fp32) ---
                xt = wp.tile([128, D], F32, tag="xt")
                nc.sync.dma_start(out=xt[:], in_=x_d[r0:r1, :])
                xsq = wp.tile([128, D], F32, tag="xsq")
                xn2 = wp.tile([128, 1], F32, tag="xn2")
                nc.scalar.activation(xsq[:], xt[:], ACTF.Square, accum_out=xn2[:])
                xsrt = wp.tile([128, 1], F32, tag="xsrt")
                nc.scalar.activation(xsrt[:], xn2[:], ACTF.Sqrt)
                xinv = wp.tile([128, 1], F32, tag="xinv")
                nc.vector.reciprocal(xinv[:], xsrt[:])
                xn = wp.tile([128, D], F32, tag="xn")
                nc.scalar.activation(xn[:], xt[:], ACTF.Copy, scale=xinv[:])

                # --- candidate tables ---
                vin = wp.tile([128, NCAND], F32, tag="vin")
                nc.sync.dma_start(out=vin[:], in_=v_d[r0:r1, :])
                jlow_i = wp.tile([128, NCAND], I32, tag="jlowi")
                nc.vector.tensor_scalar(jlow_i[:], vin[:].bitcast(I32),
                                        _PACK_MASK, None, op0=ALU.bitwise_and)
                jlow_f = wp.tile([128, NCAND], F32, tag="jlowf")
                nc.vector.tensor_copy(jlow_f[:], jlow_i[:])
                vb = wp.tile([128, NCAND], F32, tag="vb")
                nc.vector.tensor_tensor(vb[:].bitcast(I32), vin[:].bitcast(I32),
                                        mask_tab[:], op=ALU.bitwise_and)
                vb2 = wp.tile([128, NCAND], F32, tag="vb2")
                nc.vector.tensor_tensor(vb2[:].bitcast(I32), vb[:].bitcast(I32),
                                        iota_cand_i[:], op=ALU.bitwise_or)

                # --- prune to top-BSLOTS slots ---
                t12 = wp.tile([128, 16], F32, tag="t12")
                nc.vector.max(out=t12[:, 0:8], in_=vb2[:])
                vrep = wp.tile([128, NCAND], F32, tag="vrep")
                nc.vector.match_replace(out=vrep[:], in_to_replace=t12[:, 0:8],
                                        in_values=vb2[:], imm_value=_NEG_BIG)
                nc.vector.max(out=t12[:, 8:16], in_=vrep[:])
                pos_i = wp.tile([128, BSLOTS], I32, tag="posi")
                nc.vector.tensor_scalar(pos_i[:], t12[:, :BSLOTS].bitcast(I32),
                                        NCAND - 1, None, op0=ALU.bitwise_and)
                pos_f = wp.tile([128, BSLOTS], F32, tag="posf")
                nc.vector.tensor_copy(pos_f[:], pos_i[:])

                # --- winner slot pid via one-hot; core-base arithmetically ---
                j_f = wp.tile([128, BSLOTS], F32, tag="jf")
                ohmul = wp.tile([128, NCAND], F32, tag="ohmul")
                for w in range(BSLOTS):
                    nc.vector.scalar_tensor_tensor(
                        ohmul[:], iota_cand_f[:], pos_f[:, w:w + 1],
                        jlow_f[:], op0=ALU.is_equal, op1=ALU.mult)
                    nc.vector.tensor_reduce(j_f[:, w:w + 1], ohmul[:],
                                            axis=AX.X, op=ALU.add)
                cb_i = wp.tile([128, BSLOTS], I32, tag="cbi")
                nc.vector.tensor_scalar(cb_i[:], pos_i[:], ~7, None,
                                        op0=ALU.bitwise_and)   # 8*core
                cb_f = wp.tile([128, BSLOTS], F32, tag="cbf")
                nc.vector.tensor_copy(cb_f[:], cb_i[:])

                # --- quad row R = NSLOT*core + pid ---
                R_f = wp.tile([128, BSLOTS], F32, tag="rf")
                nc.vector.tensor_scalar(R_f[:], cb_f[:], float(NSLOT // 8),
                                        None, op0=ALU.mult)
                nc.vector.tensor_tensor(R_f[:], R_f[:], j_f[:], op=ALU.add)
                R_i = wp.tile([128, BSLOTS], I32, tag="ri")
                nc.vector.tensor_copy(R_i[:], R_f[:])

                # value rows rowv[m*12+s] = 4*R_s + m
                rowv = wp.tile([128, 4, BSLOTS], F32, tag="rowv")
                nc.vector.tensor_scalar(
                    rowv[:], R_f[:].unsqueeze(1).to_broadcast(
                        [128, 4, BSLOTS]),
                    4.0, None, op0=ALU.mult)
                nc.vector.tensor_tensor(
                    rowv[:].rearrange("p m s -> p (m s)"),
                    rowv[:].rearrange("p m s -> p (m s)"), m_tab[:],
                    op=ALU.add)

                # --- gather quad rows (chunked) + exact rescore ---
                sco = wp.tile([128, 4, BSLOTS], F32, tag="sco")
                xb = xn[:].unsqueeze(1).to_broadcast([128, GCH, D])
                for s0 in range(0, BSLOTS, GCH):
                    g = gp.tile([128, GCH, 4 * D], F32, tag=f"g{s0 // GCH}")
                    for s in range(GCH):
                        nc.gpsimd.indirect_dma_start(
                            out=g[:, s, :], out_offset=None, in_=k4_d[:],
                            in_offset=IndirectOffsetOnAxis(
                                ap=R_i[:, s0 + s:s0 + s + 1], axis=0))
                    gm = g[:].rearrange("p s (m d) -> p s m d", d=D)
                    for m in range(4):
                        prod = gp.tile([128, GCH, D], F32, tag=f"prod{m % 2}")
                        nc.vector.tensor_tensor(
                            prod[:], gm[:, :, m, :], xb, op=ALU.mult)
                        if m < 2:
                            nc.vector.tensor_reduce(
                                sco[:, m, s0:s0 + GCH], prod[:], axis=AX.X,
                                op=ALU.add)
                        else:
                            # per-slot reduction on the idle ACT engine
                            for s in range(GCH):
                                nc.scalar.activation(
                                    prod[:, s, :], prod[:, s, :], ACTF.Copy,
                                    accum_out=sco[:, m, s0 + s:s0 + s + 1])

                # --- exact top-8 of the 48 members ---
                scof = sco[:].rearrange("p m s -> p (m s)")
                top8 = wp.tile([128, 8], F32, tag="top8")
                nc.vector.max(out=top8[:], in_=scof)
                pos8 = wp.tile([128, 8], U32, tag="pos8")
                nc.vector.max_index(pos8[:], top8[:], scof)
                pos8f = wp.tile([128, 8], F32, tag="pos8f")
                nc.vector.tensor_copy(pos8f[:], pos8[:])

                # --- softmax ---
                sh = wp.tile([128, 8], F32, tag="sh")
                nc.vector.tensor_tensor(sh[:], top8[:],
                                        top8[:, 0:1].to_broadcast([128, 8]),
                                        op=ALU.subtract)
                ex = wp.tile([128, 8], F32, tag="ex")
                nc.scalar.activation(ex[:], sh[:], ACTF.Exp)
                es = wp.tile([128, 1], F32, tag="es")
                nc.vector.tensor_reduce(es[:], ex[:], axis=AX.X, op=ALU.add)
                esr = wp.tile([128, 1], F32, tag="esr")
                nc.vector.reciprocal(esr[:], es[:])
                wgt = wp.tile([128, 8], F32, tag="wgt")
                nc.vector.tensor_tensor(wgt[:], ex[:],
                                        esr[:].to_broadcast([128, 8]),
                                        op=ALU.mult)

                # --- winner value rows via one-hot over member index ---
                winr = wp.tile([128, 8], F32, tag="winr")
                ohm = wp.tile([128, nmemb], F32, tag="ohm")
                rowvf = rowv[:].rearrange("p m s -> p (m s)")
                for w in range(8):
                    nc.vector.scalar_tensor_tensor(
                        ohm[:], iota_m_f[:], pos8f[:, w:w + 1], rowvf,
                        op0=ALU.is_equal, op1=ALU.mult)
                    nc.vector.tensor_reduce(winr[:, w:w + 1], ohm[:], axis=AX.X,
                                            op=ALU.add)
                winr_i = wp.tile([128, 8], I32, tag="winri")
                nc.vector.tensor_copy(winr_i[:], winr[:])

                # --- gather value rows, weighted sum ---
                vg = gp.tile([128, 8, D], F32, tag="vg")
                for k in range(8):
                    nc.gpsimd.indirect_dma_start(
                        out=vg[:, k, :], out_offset=None, in_=val4_d[:],
                        in_offset=IndirectOffsetOnAxis(ap=winr_i[:, k:k + 1],
                                                       axis=0))
                vw = gp.tile([128, 8, D], F32, tag="vw")
                nc.vector.tensor_tensor(
                    vw[:], vg[:],
                    wgt[:].unsqueeze(2).to_broadcast([128, 8, D]), op=ALU.mult)
                ot = wp.tile([128, D], F32, tag="ot")
                nc.vector.tensor_reduce(ot[:], vw[:].rearrange("p k d -> p d k"),
                                        axis=AX.X, op=ALU.add)
                nc.sync.dma_start(out=out_d[r0:r1, :], in_=ot[:])

    nc.compile()
    return nc


# --------------------------------------------------------------------------
# Host orchestration
# --------------------------------------------------------------------------

_CACHE = {}


def _get_programs():
    if "A" not in _CACHE:
        _CACHE["A"] = build_dispatch_a()
    if "B" not in _CACHE:
        _CACHE["B"] = build_dispatch_b(B // NCORES)
    return _CACHE["A"], _CACHE["B"]


def _quad_cols():
    """P(S, m) per core; -1 for pad members."""
    P = -np.ones((NSLOT, 4), np.int64)
    S = np.arange(3072)
    G, j = S // 512, S % 512
    for m in range(4):
        P[S, m] = 2048 * G + 512 * m + j
    Sl = np.arange(3072, NSLOT)
    P[Sl, 0] = 12288 + (Sl - 3072)
    return P


_P4 = _quad_cols()                               # [NSLOT, 4]
_P4_VALID = _P4 >= 0
_P4G = (np.arange(NCORES)[:, None, None] * NLOC + _P4[None])  # [8, NSLOT, 4]


def kernel(x, keys, values, top_k):
    assert int(top_k) == TOPK
    x = np.ascontiguousarray(np.asarray(x, dtype=np.float32))
    keys = np.asarray(keys, dtype=np.float32)
    values = np.asarray(values, dtype=np.float32)
    assert x.shape == (B, D) and keys.shape == (N, D) and values.shape == (N, D)

    keys_sh = np.zeros((NCORES * NLOC, D), dtype=np.float32)
    keys_sh[:N] = keys
    values_sh = np.zeros((NCORES * NLOC, D), dtype=np.float32)
    values_sh[:N] = values

    nc_a, nc_b = _get_programs()
    core_ids = list(range(NCORES))

    # ---- dispatch A ----
    in_maps_a = [
        {"x": x,
         "keys": np.ascontiguousarray(keys_sh[c * NLOC:(c + 1) * NLOC])}
        for c in range(NCORES)
    ]
    t0 = time.perf_counter()
    res_a = run_bass_kernel_spmd(nc_a, in_maps_a, core_ids)
    t1 = time.perf_counter()
    cand = np.concatenate([res_a.results[c]["cand"] for c in range(NCORES)],
                          axis=1)  # [B, 64]

    # ---- host glue: normalized quad tables ----
    norms = np.sqrt(np.einsum("nd,nd->n", keys_sh, keys_sh))
    kn_sh = keys_sh / np.maximum(norms, 1e-12)[:, None]
    valid = np.broadcast_to(_P4_VALID[None], _P4G.shape)
    k4 = np.zeros((NCORES, NSLOT, 4, D), dtype=np.float32)
    k4[valid] = kn_sh[_P4G[valid]]
    k4 = np.ascontiguousarray(k4.reshape(NQUAD, 4 * D))
    v4 = np.zeros((NCORES, NSLOT, 4, D), dtype=np.float32)
    v4[valid] = values_sh[_P4G[valid]]
    v4 = np.ascontiguousarray(v4.reshape(NPADR, D))

    # ---- dispatch B ----
    bs = B // NCORES
    in_maps_b = [
        {
            "vals": np.ascontiguousarray(cand[c * bs:(c + 1) * bs]),
            "x": np.ascontiguousarray(x[c * bs:(c + 1) * bs]),
            "keysn4": k4,
            "values4": v4,
        }
        for c in range(NCORES)
    ]
    t2 = time.perf_counter()
    res_b = run_bass_kernel_spmd(nc_b, in_maps_b, core_ids)
    t3 = time.perf_counter()
    out = np.concatenate([res_b.results[c]["out"] for c in range(NCORES)],
                         axis=0)
    kernel.last_walltimes = (t1 - t0, t3 - t2)
    return out.astype(np.float32)


# revision 17
# speedup vs baseline: 1.3179x; 1.0757x over previous
"""Distributed cosine-similarity kNN retrieval (EpisodicSDM) on 8 Trainium2 cores.

Reference computation:
    x_norm = normalize(x); k_norm = normalize(keys)
    scores = x_norm @ k_norm.T               [B, N]
    top_vals, top_idx = top_k(scores, 8)
    out = sum_k softmax(top_vals)_k * values[top_idx_k]

Two SPMD dispatches, no collectives.

Dispatch A (keys sharded along N: 12544 columns/core, all queries):
  - key prep: row inv-norms (squares alternate ACT/DVE to balance engines),
    diag(1/||k||) built on GPSIMD, fp32 matmul transpose-and-scale ->
    kT bf16 [128, 2, 12544]
  - coarse scores in bf16 (fp32 PSUM accumulate), 512-wide matmuls
  - quad-slot reduction: 6 full groups of 2048 cols; within group,
    col P = 2048G + 512m + j belongs to slot S = 512G + j (member m).
    fold1 (DVE): max(pa, pb) pairs members (m, m+2); fold2 (DVE):
    max of m1 halves pairs member parity -> m2[S] = max over 4 members.
    Leftover cols 12288..12544 are single-member slots (ACT copy).
  - qtile 0's groups are interleaved with the key-prep blocks so the main
    pipeline starts as soon as kT block 0 exists.
  - pack slot id (12 bits, linear) into the f32 mantissa, max8 -> per-core
    top-8 packed (value, slot), tie-free.
  -> output: packed candidates [B, 8] per core.

Host glue: normalize keys (numpy), build the quad-layout rescore tables:
  keysn4[c*NSLOT + S] = 4 normalized member key rows (pads zero),
  values4[4*(c*NSLOT+S) + m] = member value rows.

Dispatch B (queries sharded, 256 per core):
  - re-pack candidates by position, top-12 slots of 64 via
    max8 + match_replace + max8
  - slot -> quad row R = NSLOT*(pos>>3) + pid; ONE indirect gather per slot
    fetches all 4 member rows (4096B contiguous)
  - exact fp32 rescore of 48 members via fused tensor_tensor_reduce,
    top-8, softmax, value rows 4R+m gathered, weighted sum.
"""

import sys
import time

_TRN_REPO = "/opt/trn_rl_repo"
if _TRN_REPO not in sys.path:
    sys.path.insert(0, _TRN_REPO)

import numpy as np

import concourse.bass as bass
import concourse.mybir as mybir
import concourse.tile as tile
from concourse import bacc
from concourse.bass import IndirectOffsetOnAxis
from concourse.bass_utils import run_bass_kernel_spmd
from concourse.masks import make_identity

F32 = mybir.dt.float32
BF16 = mybir.dt.bfloat16
I32 = mybir.dt.int32
U32 = mybir.dt.uint32
ALU = mybir.AluOpType
ACTF = mybir.ActivationFunctionType
AX = mybir.AxisListType

# ---- problem constants ----
B = 2048
D = 256
N = 100000
TOPK = 8
NCORES = 8
NLOC = 12544              # key columns per core; 8*12544 = 100352 >= N
NSLOT = 3328              # 6*512 quad slots + 256 single-member leftover
NGRPF = 6                 # full quad groups of 2048 columns
NQUAD = NCORES * NSLOT    # rescore-table quad rows
NPADR = 4 * NQUAD         # member rows in values4
BSLOTS = 12               # slots rescored per query after the merge
NCAND = NCORES * 8

_PACK_MASK = 0xFFF
_NEG_BIG = -3.0e38


# --------------------------------------------------------------------------
# Dispatch A
# --------------------------------------------------------------------------

def build_dispatch_a(bq=B, dbg=False):
    qtiles = bq // 128
    kchunks = NLOC // 128          # 98

    nc = bacc.Bacc("TRN2", target_bir_lowering=False, debug=dbg)
    x_d = nc.dram_tensor("x", [bq, D], F32, kind="ExternalInput").ap()
    k_d = nc.dram_tensor("keys", [NLOC, D], F32, kind="ExternalInput").ap()
    out_d = nc.dram_tensor("cand", [bq, 8], F32, kind="ExternalOutput").ap()

    with tile.TileContext(nc) as tc:
        with (
            tc.tile_pool(name="const", bufs=1) as constp,
            tc.tile_pool(name="kprep", bufs=4) as kprep,
            tc.tile_pool(name="big", bufs=1) as bigp,
            tc.tile_pool(name="xp", bufs=2) as xp,
            tc.tile_pool(name="sp", bufs=4) as sp,
            tc.tile_pool(name="m1p", bufs=2) as m1p,
            tc.tile_pool(name="m2p", bufs=2) as m2p,
            tc.tile_pool(name="ps", bufs=2, space="PSUM") as psp,
        ):
            identb = constp.tile([128, 128], BF16)
            make_identity(nc, identb[:])
            identf = constp.tile([128, 128], F32)
            make_identity(nc, identf[:])
            eps = constp.tile([128, 1], F32)
            nc.gpsimd.memset(eps[:], 1e-30)
            iota_pack = constp.tile([128, NSLOT], I32)
            nc.gpsimd.iota(iota_pack[:], pattern=[[1, NSLOT]], base=0,
                           channel_multiplier=0)
            maskc = constp.tile([128, 1], I32)
            nc.gpsimd.memset(maskc[:], -(_PACK_MASK + 1))

            kT = bigp.tile([128, 2, NLOC], BF16)
            xT = bigp.tile([128, 2, bq], BF16)

            # ---- x prep: normalize, cast, transpose into xT ----
            for qt in range(qtiles):
                xt = xp.tile([128, D], F32, tag="xt")
                nc.sync.dma_start(out=xt[:], in_=x_d[qt * 128:(qt + 1) * 128, :])
                xsq = xp.tile([128, D], F32, tag="xsq")
                nc.vector.tensor_tensor(xsq[:], xt[:], xt[:], op=ALU.mult)
                xn2 = xp.tile([128, 1], F32, tag="xn2")
                nc.vector.tensor_reduce(xn2[:], xsq[:], axis=AX.X, op=ALU.add)
                xsrt = xp.tile([128, 1], F32, tag="xsrt")
                nc.scalar.activation(xsrt[:], xn2[:], ACTF.Sqrt)
                xinv = xp.tile([128, 1], F32, tag="xinv")
                nc.vector.reciprocal(xinv[:], xsrt[:])
                xnb = xp.tile([128, D], BF16, tag="xnb")
                nc.scalar.activation(xnb[:], xt[:], ACTF.Copy, scale=xinv[:])
                pt = psp.tile([128, 1024], F32, tag="pa")
                for c in range(2):
                    nc.tensor.matmul(pt[:, c * 128:(c + 1) * 128],
                                     lhsT=xnb[:, c * 128:(c + 1) * 128],
                                     rhs=identb[:], start=True, stop=True)
                nc.scalar.activation(
                    xT[:, :, qt * 128:(qt + 1) * 128], pt[:, :256], ACTF.Copy)

            # ---- helpers ----
            def key_prep_chunks(chunks):
                """Prep a run of 128-key chunks (multiple of 4 or the tail)."""
                for g4s in range(0, len(chunks), 4):
                    grp = chunks[g4s:g4s + 4]
                    nch = len(grp)
                    ktfs = []
                    kn2g = kprep.tile([128, 4], F32, tag="kn2g")
                    for i, ch in enumerate(grp):
                        ktf = kprep.tile([128, D], F32, tag=f"ktf{i}")
                        nc.sync.dma_start(out=ktf[:],
                                          in_=k_d[ch * 128:(ch + 1) * 128, :])
                        if ch % 4 == 0:
                            ksq = kprep.tile([128, D], F32, tag=f"ksq{i % 2}")
                            nc.scalar.activation(ksq[:], ktf[:], ACTF.Square,
                                                 accum_out=kn2g[:, i:i + 1])
                        else:
                            ksq = kprep.tile([128, D], F32, tag=f"ksq{i % 2}")
                            nc.vector.tensor_tensor(ksq[:], ktf[:], ktf[:],
                                                    op=ALU.mult)
                            nc.vector.tensor_reduce(kn2g[:, i:i + 1], ksq[:],
                                                    axis=AX.X, op=ALU.add)
                        ktfs.append(ktf)
                    ksrt = kprep.tile([128, 4], F32, tag="ksrt")
                    nc.scalar.activation(ksrt[:, :nch], kn2g[:, :nch],
                                         ACTF.Sqrt, bias=eps[:])
                    kinvg = kprep.tile([128, 4], F32, tag="kinvg")
                    nc.vector.reciprocal(kinvg[:, :nch], ksrt[:, :nch])
                    for i, ch in enumerate(grp):
                        diag = kprep.tile([128, 128], F32, tag=f"diag{i}")
                        nc.gpsimd.tensor_tensor(
                            diag[:], identf[:],
                            kinvg[:, i:i + 1].to_broadcast([128, 128]),
                            op=ALU.mult)
                        pt = psp.tile([128, 1024], F32, tag="pb")
                        for c in range(2):
                            nc.tensor.matmul(
                                pt[:, c * 128:(c + 1) * 128],
                                lhsT=ktfs[i][:, c * 128:(c + 1) * 128],
                                rhs=diag[:], start=True, stop=True)
                        if ch % 4 == 1:
                            nc.vector.tensor_copy(
                                kT[:, :, ch * 128:(ch + 1) * 128],
                                pt[:, :256])
                        else:
                            nc.scalar.activation(
                                kT[:, :, ch * 128:(ch + 1) * 128],
                                pt[:, :256], ACTF.Copy)

            NBF = 6   # groups using bf16 folds (ACT copies both PSUM banks)

            def main_group(qt, g, m2):
                qs = slice(qt * 128, (qt + 1) * 128)
                base = 2048 * g
                pa = psp.tile([128, 1024], F32, tag="pa")
                pb = psp.tile([128, 1024], F32, tag="pb")
                for half, pp in ((0, pa), (1, pa), (2, pb), (3, pb)):
                    dst = pp[:, (half % 2) * 512:(half % 2 + 1) * 512]
                    cs = base + half * 512
                    for c in range(2):
                        nc.tensor.matmul(
                            dst, lhsT=xT[:, c, qs],
                            rhs=kT[:, c, cs:cs + 512],
                            start=(c == 0), stop=(c == 1))
                if g < NBF:
                    # bf16 path: ACT casts both banks, DVE folds at 2x
                    sa = sp.tile([128, 1024], BF16, tag="sa")
                    nc.scalar.activation(sa[:], pa[:], ACTF.Copy)
                    sb = sp.tile([128, 1024], BF16, tag="sb")
                    nc.scalar.activation(sb[:], pb[:], ACTF.Copy)
                    m1b = m1p.tile([128, 1024], BF16, tag="m1b")
                    nc.vector.tensor_tensor(m1b[:], sa[:], sb[:], op=ALU.max)
                    nc.vector.tensor_tensor(
                        m2[:, 512 * g:512 * (g + 1)],
                        m1b[:, :512], m1b[:, 512:], op=ALU.max)
                else:
                    stmp = sp.tile([128, 1024], F32, tag="stmp")
                    nc.scalar.activation(stmp[:], pa[:], ACTF.Copy)
                    m1g = m1p.tile([128, 1024], F32, tag="m1g")
                    nc.vector.tensor_tensor(m1g[:], pb[:], stmp[:], op=ALU.max)
                    nc.vector.tensor_tensor(
                        m2[:, 512 * g:512 * (g + 1)],
                        m1g[:, :512], m1g[:, 512:], op=ALU.max)

            def main_leftover(qt, m2):
                qs = slice(qt * 128, (qt + 1) * 128)
                pa = psp.tile([128, 1024], F32, tag="pa")
                for c in range(2):
                    nc.tensor.matmul(pa[:, :256], lhsT=xT[:, c, qs],
                                     rhs=kT[:, c, 12288:12544],
                                     start=(c == 0), stop=(c == 1))
                nc.scalar.activation(m2[:, 3072:3328], pa[:, :256], ACTF.Copy)

            def main_tail(qt, m2):
                nc.vector.scalar_tensor_tensor(
                    m2[:].bitcast(I32), m2[:].bitcast(I32),
                    maskc[:], iota_pack[:],
                    op0=ALU.bitwise_and, op1=ALU.bitwise_or)
                top = m2p.tile([128, 8], F32, tag="top")
                nc.vector.max(out=top[:], in_=m2[:])
                nc.sync.dma_start(out=out_d[qt * 128:(qt + 1) * 128, :],
                                  in_=top[:])

            # ---- qtiles 0..3 interleaved with key prep blocks ----
            NIL = 0
            m2s = []
            for _i in range(NIL):
                m2q = m2p.tile([128, NSLOT], F32, tag="m2")
                m2s.append(m2q)

            key_prep_chunks(list(range(0, 16)))
            for g in range(NGRPF):
                # overlap mains of ready block g with prep of block g+1
                nxt = (list(range(16 * (g + 1), 16 * (g + 2)))
                       if g + 1 < NGRPF else [96, 97])
                if NIL:
                    bsz = (len(nxt) + NIL - 1) // NIL
                    for q in range(NIL):
                        key_prep_chunks(nxt[bsz * q:bsz * (q + 1)])
                        main_group(q, g, m2s[q])
                else:
                    key_prep_chunks(nxt)
            for q in range(NIL):
                main_leftover(q, m2s[q])
                main_tail(q, m2s[q])

            # ---- remaining qtiles ----
            for qt in range(NIL, qtiles):
                m2 = m2p.tile([128, NSLOT], F32, tag="m2")
                for g in range(NGRPF):
                    main_group(qt, g, m2)
                main_leftover(qt, m2)
                main_tail(qt, m2)

    nc.compile()
    return nc


# --------------------------------------------------------------------------
# Dispatch B
# --------------------------------------------------------------------------

def build_dispatch_b(bq_slice, dbg=False):
    qtiles = bq_slice // 128
    nmemb = BSLOTS * 4            # 48 rescored members
    GCH = 6                       # slots per gather chunk tile

    nc = bacc.Bacc("TRN2", target_bir_lowering=False, debug=dbg)
    v_d = nc.dram_tensor("vals", [bq_slice, NCAND], F32,
                         kind="ExternalInput").ap()
    x_d = nc.dram_tensor("x", [bq_slice, D], F32, kind="ExternalInput").ap()
    k4_d = nc.dram_tensor("keysn4", [NQUAD, 4 * D], F32,
                          kind="ExternalInput").ap()
    val4_d = nc.dram_tensor("values4", [NPADR, D], F32,
                            kind="ExternalInput").ap()
    out_d = nc.dram_tensor("out", [bq_slice, D], F32, kind="ExternalOutput").ap()

    with tile.TileContext(nc) as tc:
        with (
            tc.tile_pool(name="const", bufs=1) as constp,
            tc.tile_pool(name="wp", bufs=2) as wp,
            tc.tile_pool(name="gp", bufs=2) as gp,
        ):
            iota_cand_i = constp.tile([128, NCAND], I32)
            nc.gpsimd.iota(iota_cand_i[:], pattern=[[1, NCAND]], base=0,
                           channel_multiplier=0)
            iota_cand_f = constp.tile([128, NCAND], F32)
            nc.gpsimd.tensor_copy(iota_cand_f[:], iota_cand_i[:])
            mask_tab = constp.tile([128, NCAND], I32)
            nc.gpsimd.memset(mask_tab[:], -(_PACK_MASK + 1))
            # member-major tables for the 48 rescored entries: idx = m*12+s
            m_tab = constp.tile([128, nmemb], F32)      # member id m
            nc.gpsimd.iota(m_tab[:], pattern=[[1, 4], [0, BSLOTS]], base=0,
                           channel_multiplier=0,
                           allow_small_or_imprecise_dtypes=True)
            iota_m_f = constp.tile([128, nmemb], F32)   # 0..47
            nc.gpsimd.iota(iota_m_f[:], pattern=[[1, nmemb]], base=0,
                           channel_multiplier=0,
                           allow_small_or_imprecise_dtypes=True)

            for qt in range(qtiles):
                r0, r1 = qt * 128, (qt + 1) * 128

                # --- x_norm (exact fp32) ---
                xt = wp.tile([128, D], F32, tag="xt")
                nc.sync.dma_start(out=xt[:], in_=x_d[r0:r1, :])
                xsq = wp.tile([128, D], F32, tag="xsq")
                xn2 = wp.tile([128, 1], F32, tag="xn2")
                nc.scalar.activation(xsq[:], xt[:], ACTF.Square, accum_out=xn2[:])
                xsrt = wp.tile([128, 1], F32, tag="xsrt")
                nc.scalar.activation(xsrt[:], xn2[:], ACTF.Sqrt)
                xinv = wp.tile([128, 1], F32, tag="xinv")
                nc.vector.reciprocal(xinv[:], xsrt[:])
                xn = wp.tile([128, D], F32, tag="xn")
                nc.scalar.activation(xn[:], xt[:], ACTF.Copy, scale=xinv[:])

                # --- candidate tables ---
                vin = wp.tile([128, NCAND], F32, tag="vin")
                nc.sync.dma_start(out=vin[:], in_=v_d[r0:r1, :])
                jlow_i = wp.tile([128, NCAND], I32, tag="jlowi")
                nc.vector.tensor_scalar(jlow_i[:], vin[:].bitcast(I32),
                                        _PACK_MASK, None, op0=ALU.bitwise_and)
                jlow_f = wp.tile([128, NCAND], F32, tag="jlowf")
                nc.vector.tensor_copy(jlow_f[:], jlow_i[:])
                vb = wp.tile([128, NCAND], F32, tag="vb")
                nc.vector.tensor_tensor(vb[:].bitcast(I32), vin[:].bitcast(I32),
                                        mask_tab[:], op=ALU.bitwise_and)
                vb2 = wp.tile([128, NCAND], F32, tag="vb2")
                nc.vector.tensor_tensor(vb2[:].bitcast(I32), vb[:].bitcast(I32),
                                        iota_cand_i[:], op=ALU.bitwise_or)

                # --- prune to top-BSLOTS slots ---
                t12 = wp.tile([128, 16], F32, tag="t12")
                nc.vector.max(out=t12[:, 0:8], in_=vb2[:])
                vrep = wp.tile([128, NCAND], F32, tag="vrep")
                nc.vector.match_replace(out=vrep[:], in_to_replace=t12[:, 0:8],
                                        in_values=vb2[:], imm_value=_NEG_BIG)
                nc.vector.max(out=t12[:, 8:16], in_=vrep[:])
                pos_i = wp.tile([128, BSLOTS], I32, tag="posi")
                nc.vector.tensor_scalar(pos_i[:], t12[:, :BSLOTS].bitcast(I32),
                                        NCAND - 1, None, op0=ALU.bitwise_and)
                pos_f = wp.tile([128, BSLOTS], F32, tag="posf")
                nc.vector.tensor_copy(pos_f[:], pos_i[:])

                # --- winner slot pid via one-hot; core-base arithmetically ---
                j_f = wp.tile([128, BSLOTS], F32, tag="jf")
                ohmul = wp.tile([128, NCAND], F32, tag="ohmul")
                for w in range(BSLOTS):
                    nc.vector.scalar_tensor_tensor(
                        ohmul[:], iota_cand_f[:], pos_f[:, w:w + 1],
                        jlow_f[:], op0=ALU.is_equal, op1=ALU.mult)
                    nc.vector.tensor_reduce(j_f[:, w:w + 1], ohmul[:],
                                            axis=AX.X, op=ALU.add)
                cb_i = wp.tile([128, BSLOTS], I32, tag="cbi")
                nc.vector.tensor_scalar(cb_i[:], pos_i[:], ~7, None,
                                        op0=ALU.bitwise_and)   # 8*core
                cb_f = wp.tile([128, BSLOTS], F32, tag="cbf")
                nc.vector.tensor_copy(cb_f[:], cb_i[:])

                # --- quad row R = NSLOT*core + pid ---
                R_f = wp.tile([128, BSLOTS], F32, tag="rf")
                nc.vector.tensor_scalar(R_f[:], cb_f[:], float(NSLOT // 8),
                                        None, op0=ALU.mult)
                nc.vector.tensor_tensor(R_f[:], R_f[:], j_f[:], op=ALU.add)
                R_i = wp.tile([128, BSLOTS], I32, tag="ri")
                nc.vector.tensor_copy(R_i[:], R_f[:])

                # value rows rowv[m*12+s] = 4*R_s + m
                rowv = wp.tile([128, 4, BSLOTS], F32, tag="rowv")
                nc.vector.tensor_scalar(
                    rowv[:], R_f[:].unsqueeze(1).to_broadcast(
                        [128, 4, BSLOTS]),
                    4.0, None, op0=ALU.mult)
                nc.vector.tensor_tensor(
                    rowv[:].rearrange("p m s -> p (m s)"),
                    rowv[:].rearrange("p m s -> p (m s)"), m_tab[:],
                    op=ALU.add)

                # --- gather quad rows (chunked) + exact rescore ---
                sco = wp.tile([128, 4, BSLOTS], F32, tag="sco")
                xb = xn[:].unsqueeze(1).to_broadcast([128, GCH, D])
                for s0 in range(0, BSLOTS, GCH):
                    g = gp.tile([128, GCH, 4 * D], F32, tag=f"g{s0 // GCH}")
                    for s in range(GCH):
                        nc.gpsimd.indirect_dma_start(
                            out=g[:, s, :], out_offset=None, in_=k4_d[:],
                            in_offset=IndirectOffsetOnAxis(
                                ap=R_i[:, s0 + s:s0 + s + 1], axis=0))
                    gm = g[:].rearrange("p s (m d) -> p s m d", d=D)
                    for m in range(4):
                        prod = gp.tile([128, GCH, D], F32, tag=f"prod{m % 2}")
                        nc.vector.tensor_tensor(
                            prod[:], gm[:, :, m, :], xb, op=ALU.mult)
                        if m < 2:
                            nc.vector.tensor_reduce(
                                sco[:, m, s0:s0 + GCH], prod[:], axis=AX.X,
                                op=ALU.add)
                        else:
                            # per-slot reduction on the idle ACT engine
                            for s in range(GCH):
                                nc.scalar.activation(
                                    prod[:, s, :], prod[:, s, :], ACTF.Copy,
                                    accum_out=sco[:, m, s0 + s:s0 + s + 1])

                # --- exact top-8 of the 48 members ---
                scof = sco[:].rearrange("p m s -> p (m s)")
                top8 = wp.tile([128, 8], F32, tag="top8")
                nc.vector.max(out=top8[:], in_=scof)
                pos8 = wp.tile([128, 8], U32, tag="pos8")
                nc.vector.max_index(pos8[:], top8[:], scof)
                pos8f = wp.tile([128, 8], F32, tag="pos8f")
                nc.vector.tensor_copy(pos8f[:], pos8[:])

                # --- softmax ---
                sh = wp.tile([128, 8], F32, tag="sh")
                nc.vector.tensor_tensor(sh[:], top8[:],
                                        top8[:, 0:1].to_broadcast([128, 8]),
                                        op=ALU.subtract)
                ex = wp.tile([128, 8], F32, tag="ex")
                nc.scalar.activation(ex[:], sh[:], ACTF.Exp)
                es = wp.tile([128, 1], F32, tag="es")
                nc.vector.tensor_reduce(es[:], ex[:], axis=AX.X, op=ALU.add)
                esr = wp.tile([128, 1], F32, tag="esr")
                nc.vector.reciprocal(esr[:], es[:])
                wgt = wp.tile([128, 8], F32, tag="wgt")
                nc.vector.tensor_tensor(wgt[:], ex[:],
                                        esr[:].to_broadcast([128, 8]),
                                        op=ALU.mult)

                # --- winner value rows via one-hot over member index ---
                winr = wp.tile([128, 8], F32, tag="winr")
                ohm = wp.tile([128, nmemb], F32, tag="ohm")
                rowvf = rowv[:].rearrange("p m s -> p (m s)")
                for w in range(8):
                    nc.vector.scalar_tensor_tensor(
                        ohm[:], iota_m_f[:], pos8f[:, w:w + 1], rowvf,
                        op0=ALU.is_equal, op1=ALU.mult)
                    nc.vector.tensor_reduce(winr[:, w:w + 1], ohm[:], axis=AX.X,
                                            op=ALU.add)
                winr_i = wp.tile([128, 8], I32, tag="winri")
                nc.vector.tensor_copy(winr_i[:], winr[:])

                # --- gather value rows, weighted sum ---
                vg = gp.tile([128, 8, D], F32, tag="vg")
                for k in range(8):
                    nc.gpsimd.indirect_dma_start(
                        out=vg[:, k, :], out_offset=None, in_=val4_d[:],
                        in_offset=IndirectOffsetOnAxis(ap=winr_i[:, k:k + 1],
                                                       axis=0))
                vw = gp.tile([128, 8, D], F32, tag="vw")
                nc.vector.tensor_tensor(
                    vw[:], vg[:],
                    wgt[:].unsqueeze(2).to_broadcast([128, 8, D]), op=ALU.mult)
                ot = wp.tile([128, D], F32, tag="ot")
                nc.vector.tensor_reduce(ot[:], vw[:].rearrange("p k d -> p d k"),
                                        axis=AX.X, op=ALU.add)
                nc.sync.dma_start(out=out_d[r0:r1, :], in_=ot[:])

    nc.compile()
    return nc


# --------------------------------------------------------------------------
# Host orchestration
# --------------------------------------------------------------------------

_CACHE = {}


def _get_programs():
    if "A" not in _CACHE:
        _CACHE["A"] = build_dispatch_a()
    if "B" not in _CACHE:
        _CACHE["B"] = build_dispatch_b(B // NCORES)
    return _CACHE["A"], _CACHE["B"]


def _quad_cols():
    """P(S, m) per core; -1 for pad members."""
    P = -np.ones((NSLOT, 4), np.int64)
    S = np.arange(3072)
    G, j = S // 512, S % 512
    for m in range(4):
        P[S, m] = 2048 * G + 512 * m + j
    Sl = np.arange(3072, NSLOT)
    P[Sl, 0] = 12288 + (Sl - 3072)
    return P


_P4 = _quad_cols()                               # [NSLOT, 4]
_P4_VALID = _P4 >= 0
_P4G = (np.arange(NCORES)[:, None, None] * NLOC + _P4[None])  # [8, NSLOT, 4]


def kernel(x, keys, values, top_k):
    assert int(top_k) == TOPK
    x = np.ascontiguousarray(np.asarray(x, dtype=np.float32))
    keys = np.asarray(keys, dtype=np.float32)
    values = np.asarray(values, dtype=np.float32)
    assert x.shape == (B, D) and keys.shape == (N, D) and values.shape == (N, D)

    keys_sh = np.zeros((NCORES * NLOC, D), dtype=np.float32)
    keys_sh[:N] = keys
    values_sh = np.zeros((NCORES * NLOC, D), dtype=np.float32)
    values_sh[:N] = values

    nc_a, nc_b = _get_programs()
    core_ids = list(range(NCORES))

    # ---- dispatch A ----
    in_maps_a = [
        {"x": x,
         "keys": np.ascontiguousarray(keys_sh[c * NLOC:(c + 1) * NLOC])}
        for c in range(NCORES)
    ]
    t0 = time.perf_counter()
    res_a = run_bass_kernel_spmd(nc_a, in_maps_a, core_ids)
    t1 = time.perf_counter()
    cand = np.concatenate([res_a.results[c]["cand"] for c in range(NCORES)],
                          axis=1)  # [B, 64]

    # ---- host glue: normalized quad tables ----
    norms = np.sqrt(np.einsum("nd,nd->n", keys_sh, keys_sh))
    kn_sh = keys_sh / np.maximum(norms, 1e-12)[:, None]
    valid = np.broadcast_to(_P4_VALID[None], _P4G.shape)
    k4 = np.zeros((NCORES, NSLOT, 4, D), dtype=np.float32)
    k4[valid] = kn_sh[_P4G[valid]]
    k4 = np.ascontiguousarray(k4.reshape(NQUAD, 4 * D))
    v4 = np.zeros((NCORES, NSLOT, 4, D), dtype=np.float32)
    v4[valid] = values_sh[_P4G[valid]]
    v4 = np.ascontiguousarray(v4.reshape(NPADR, D))

    # ---- dispatch B ----
    bs = B // NCORES
    in_maps_b = [
        {
            "vals": np.ascontiguousarray(cand[c * bs:(c + 1) * bs]),
            "x": np.ascontiguousarray(x[c * bs:(c + 1) * bs]),
            "keysn4": k4,
            "values4": v4,
        }
        for c in range(NCORES)
    ]
    t2 = time.perf_counter()
    res_b = run_bass_kernel_spmd(nc_b, in_maps_b, core_ids)
    t3 = time.perf_counter()
    out = np.concatenate([res_b.results[c]["out"] for c in range(NCORES)],
                         axis=0)
    kernel.last_walltimes = (t1 - t0, t3 - t2)
    return out.astype(np.float32)


# revision 18
# speedup vs baseline: 1.3392x; 1.0161x over previous
"""Distributed cosine-similarity kNN retrieval (EpisodicSDM) on 8 Trainium2 cores.

Reference computation:
    x_norm = normalize(x); k_norm = normalize(keys)
    scores = x_norm @ k_norm.T               [B, N]
    top_vals, top_idx = top_k(scores, 8)
    out = sum_k softmax(top_vals)_k * values[top_idx_k]

Two SPMD dispatches, no collectives.

Dispatch A (keys sharded along N: 12544 columns/core, all queries):
  - key prep: row inv-norms (squares alternate ACT/DVE to balance engines),
    diag(1/||k||) built on GPSIMD, fp32 matmul transpose-and-scale ->
    kT bf16 [128, 2, 12544]
  - coarse scores in bf16 (fp32 PSUM accumulate), 512-wide matmuls
  - quad-slot reduction: 6 full groups of 2048 cols; within group,
    col P = 2048G + 512m + j belongs to slot S = 512G + j (member m).
    fold1 (DVE): max(pa, pb) pairs members (m, m+2); fold2 (DVE):
    max of m1 halves pairs member parity -> m2[S] = max over 4 members.
    Leftover cols 12288..12544 are single-member slots (ACT copy).
  - qtile 0's groups are interleaved with the key-prep blocks so the main
    pipeline starts as soon as kT block 0 exists.
  - pack slot id (12 bits, linear) into the f32 mantissa, max8 -> per-core
    top-8 packed (value, slot), tie-free.
  -> output: packed candidates [B, 8] per core.

Host glue: normalize keys (numpy), build the quad-layout rescore tables:
  keysn4[c*NSLOT + S] = 4 normalized member key rows (pads zero),
  values4[4*(c*NSLOT+S) + m] = member value rows.

Dispatch B (queries sharded, 256 per core):
  - re-pack candidates by position, top-12 slots of 64 via
    max8 + match_replace + max8
  - slot -> quad row R = NSLOT*(pos>>3) + pid; ONE indirect gather per slot
    fetches all 4 member rows (4096B contiguous)
  - exact fp32 rescore of 48 members via fused tensor_tensor_reduce,
    top-8, softmax, value rows 4R+m gathered, weighted sum.
"""

import sys
import time

_TRN_REPO = "/opt/trn_rl_repo"
if _TRN_REPO not in sys.path:
    sys.path.insert(0, _TRN_REPO)

import numpy as np

import concourse.bass as bass
import concourse.mybir as mybir
import concourse.tile as tile
from concourse import bacc
from concourse.bass import IndirectOffsetOnAxis
from concourse.bass_utils import run_bass_kernel_spmd
from concourse.masks import make_identity

F32 = mybir.dt.float32
BF16 = mybir.dt.bfloat16
I32 = mybir.dt.int32
U32 = mybir.dt.uint32
ALU = mybir.AluOpType
ACTF = mybir.ActivationFunctionType
AX = mybir.AxisListType

# ---- problem constants ----
B = 2048
D = 256
N = 100000
TOPK = 8
NCORES = 8
NLOC = 12544              # key columns per core; 8*12544 = 100352 >= N
NSLOT = 3328              # 6*512 quad slots + 256 single-member leftover
NGRPF = 6                 # full quad groups of 2048 columns
NQUAD = NCORES * NSLOT    # rescore-table quad rows
NPADR = 4 * NQUAD         # member rows in values4
BSLOTS = 12               # slots rescored per query after the merge
NCAND = NCORES * 8

_PACK_MASK = 0x7FFF
_NEG_BIG = -3.0e38


# --------------------------------------------------------------------------
# Dispatch A
# --------------------------------------------------------------------------

def build_dispatch_a(bq=B, dbg=False):
    qtiles = bq // 128
    kchunks = NLOC // 128          # 98

    nc = bacc.Bacc("TRN2", target_bir_lowering=False, debug=dbg)
    x_d = nc.dram_tensor("x", [bq, D], F32, kind="ExternalInput").ap()
    k_d = nc.dram_tensor("keys", [NLOC, D], F32, kind="ExternalInput").ap()
    cb_d = nc.dram_tensor("cbase", [128, 1], I32, kind="ExternalInput").ap()
    out_d = nc.dram_tensor("cand", [bq, 8], F32, kind="ExternalOutput").ap()

    with tile.TileContext(nc) as tc:
        with (
            tc.tile_pool(name="const", bufs=1) as constp,
            tc.tile_pool(name="kprep", bufs=4) as kprep,
            tc.tile_pool(name="big", bufs=1) as bigp,
            tc.tile_pool(name="xp", bufs=2) as xp,
            tc.tile_pool(name="sp", bufs=4) as sp,
            tc.tile_pool(name="m1p", bufs=2) as m1p,
            tc.tile_pool(name="m2p", bufs=2) as m2p,
            tc.tile_pool(name="ps", bufs=2, space="PSUM") as psp,
        ):
            identb = constp.tile([128, 128], BF16)
            make_identity(nc, identb[:])
            identf = constp.tile([128, 128], F32)
            make_identity(nc, identf[:])
            eps = constp.tile([128, 1], F32)
            nc.gpsimd.memset(eps[:], 1e-30)
            iota_pack = constp.tile([128, NSLOT], I32)
            nc.gpsimd.iota(iota_pack[:], pattern=[[1, NSLOT]], base=0,
                           channel_multiplier=0)
            cbase = constp.tile([128, 1], I32)
            nc.sync.dma_start(out=cbase[:], in_=cb_d[:, :])
            nc.gpsimd.tensor_tensor(iota_pack[:], iota_pack[:],
                                    cbase[:].to_broadcast([128, NSLOT]),
                                    op=ALU.add)
            maskc = constp.tile([128, 1], I32)
            nc.gpsimd.memset(maskc[:], -(_PACK_MASK + 1))

            kT = bigp.tile([128, 2, NLOC], BF16)
            xT = bigp.tile([128, 2, bq], BF16)

            # ---- x prep: normalize, cast, transpose into xT ----
            for qt in range(qtiles):
                xt = xp.tile([128, D], F32, tag="xt")
                nc.sync.dma_start(out=xt[:], in_=x_d[qt * 128:(qt + 1) * 128, :])
                xsq = xp.tile([128, D], F32, tag="xsq")
                nc.vector.tensor_tensor(xsq[:], xt[:], xt[:], op=ALU.mult)
                xn2 = xp.tile([128, 1], F32, tag="xn2")
                nc.vector.tensor_reduce(xn2[:], xsq[:], axis=AX.X, op=ALU.add)
                xsrt = xp.tile([128, 1], F32, tag="xsrt")
                nc.scalar.activation(xsrt[:], xn2[:], ACTF.Sqrt)
                xinv = xp.tile([128, 1], F32, tag="xinv")
                nc.vector.reciprocal(xinv[:], xsrt[:])
                xnb = xp.tile([128, D], BF16, tag="xnb")
                nc.scalar.activation(xnb[:], xt[:], ACTF.Copy, scale=xinv[:])
                pt = psp.tile([128, 1024], F32, tag="pa")
                for c in range(2):
                    nc.tensor.matmul(pt[:, c * 128:(c + 1) * 128],
                                     lhsT=xnb[:, c * 128:(c + 1) * 128],
                                     rhs=identb[:], start=True, stop=True)
                nc.scalar.activation(
                    xT[:, :, qt * 128:(qt + 1) * 128], pt[:, :256], ACTF.Copy)

            # ---- helpers ----
            def key_prep_chunks(chunks):
                """Prep a run of 128-key chunks (multiple of 4 or the tail)."""
                for g4s in range(0, len(chunks), 4):
                    grp = chunks[g4s:g4s + 4]
                    nch = len(grp)
                    ktfs = []
                    kn2g = kprep.tile([128, 4], F32, tag="kn2g")
                    for i, ch in enumerate(grp):
                        ktf = kprep.tile([128, D], F32, tag=f"ktf{i}")
                        nc.sync.dma_start(out=ktf[:],
                                          in_=k_d[ch * 128:(ch + 1) * 128, :])
                        if ch % 4 == 0:
                            ksq = kprep.tile([128, D], F32, tag=f"ksq{i % 2}")
                            nc.scalar.activation(ksq[:], ktf[:], ACTF.Square,
                                                 accum_out=kn2g[:, i:i + 1])
                        else:
                            ksq = kprep.tile([128, D], F32, tag=f"ksq{i % 2}")
                            nc.vector.tensor_tensor(ksq[:], ktf[:], ktf[:],
                                                    op=ALU.mult)
                            nc.vector.tensor_reduce(kn2g[:, i:i + 1], ksq[:],
                                                    axis=AX.X, op=ALU.add)
                        ktfs.append(ktf)
                    ksrt = kprep.tile([128, 4], F32, tag="ksrt")
                    nc.scalar.activation(ksrt[:, :nch], kn2g[:, :nch],
                                         ACTF.Sqrt, bias=eps[:])
                    kinvg = kprep.tile([128, 4], F32, tag="kinvg")
                    nc.vector.reciprocal(kinvg[:, :nch], ksrt[:, :nch])
                    for i, ch in enumerate(grp):
                        diag = kprep.tile([128, 128], F32, tag=f"diag{i}")
                        nc.gpsimd.tensor_tensor(
                            diag[:], identf[:],
                            kinvg[:, i:i + 1].to_broadcast([128, 128]),
                            op=ALU.mult)
                        pt = psp.tile([128, 1024], F32, tag="pb")
                        for c in range(2):
                            nc.tensor.matmul(
                                pt[:, c * 128:(c + 1) * 128],
                                lhsT=ktfs[i][:, c * 128:(c + 1) * 128],
                                rhs=diag[:], start=True, stop=True)
                        if ch % 4 == 1:
                            nc.vector.tensor_copy(
                                kT[:, :, ch * 128:(ch + 1) * 128],
                                pt[:, :256])
                        else:
                            nc.scalar.activation(
                                kT[:, :, ch * 128:(ch + 1) * 128],
                                pt[:, :256], ACTF.Copy)

            NBF = 6   # groups using bf16 folds (ACT copies both PSUM banks)

            def main_group(qt, g, m2):
                qs = slice(qt * 128, (qt + 1) * 128)
                base = 2048 * g
                pa = psp.tile([128, 1024], F32, tag="pa")
                pb = psp.tile([128, 1024], F32, tag="pb")
                for half, pp in ((0, pa), (1, pa), (2, pb), (3, pb)):
                    dst = pp[:, (half % 2) * 512:(half % 2 + 1) * 512]
                    cs = base + half * 512
                    for c in range(2):
                        nc.tensor.matmul(
                            dst, lhsT=xT[:, c, qs],
                            rhs=kT[:, c, cs:cs + 512],
                            start=(c == 0), stop=(c == 1))
                if g < NBF:
                    # bf16 path: ACT casts both banks, DVE folds at 2x
                    sa = sp.tile([128, 1024], BF16, tag="sa")
                    nc.scalar.activation(sa[:], pa[:], ACTF.Copy)
                    sb = sp.tile([128, 1024], BF16, tag="sb")
                    nc.scalar.activation(sb[:], pb[:], ACTF.Copy)
                    m1b = m1p.tile([128, 1024], BF16, tag="m1b")
                    nc.vector.tensor_tensor(m1b[:], sa[:], sb[:], op=ALU.max)
                    nc.vector.tensor_tensor(
                        m2[:, 512 * g:512 * (g + 1)],
                        m1b[:, :512], m1b[:, 512:], op=ALU.max)
                else:
                    stmp = sp.tile([128, 1024], F32, tag="stmp")
                    nc.scalar.activation(stmp[:], pa[:], ACTF.Copy)
                    m1g = m1p.tile([128, 1024], F32, tag="m1g")
                    nc.vector.tensor_tensor(m1g[:], pb[:], stmp[:], op=ALU.max)
                    nc.vector.tensor_tensor(
                        m2[:, 512 * g:512 * (g + 1)],
                        m1g[:, :512], m1g[:, 512:], op=ALU.max)

            def main_leftover(qt, m2):
                qs = slice(qt * 128, (qt + 1) * 128)
                pa = psp.tile([128, 1024], F32, tag="pa")
                for c in range(2):
                    nc.tensor.matmul(pa[:, :256], lhsT=xT[:, c, qs],
                                     rhs=kT[:, c, 12288:12544],
                                     start=(c == 0), stop=(c == 1))
                nc.scalar.activation(m2[:, 3072:3328], pa[:, :256], ACTF.Copy)

            def main_tail(qt, m2):
                nc.vector.scalar_tensor_tensor(
                    m2[:].bitcast(I32), m2[:].bitcast(I32),
                    maskc[:], iota_pack[:],
                    op0=ALU.bitwise_and, op1=ALU.bitwise_or)
                top = m2p.tile([128, 8], F32, tag="top")
                nc.vector.max(out=top[:], in_=m2[:])
                nc.sync.dma_start(out=out_d[qt * 128:(qt + 1) * 128, :],
                                  in_=top[:])

            # ---- qtiles 0..3 interleaved with key prep blocks ----
            NIL = 0
            m2s = []
            for _i in range(NIL):
                m2q = m2p.tile([128, NSLOT], F32, tag="m2")
                m2s.append(m2q)

            key_prep_chunks(list(range(0, 16)))
            for g in range(NGRPF):
                # overlap mains of ready block g with prep of block g+1
                nxt = (list(range(16 * (g + 1), 16 * (g + 2)))
                       if g + 1 < NGRPF else [96, 97])
                if NIL:
                    bsz = (len(nxt) + NIL - 1) // NIL
                    for q in range(NIL):
                        key_prep_chunks(nxt[bsz * q:bsz * (q + 1)])
                        main_group(q, g, m2s[q])
                else:
                    key_prep_chunks(nxt)
            for q in range(NIL):
                main_leftover(q, m2s[q])
                main_tail(q, m2s[q])

            # ---- remaining qtiles ----
            for qt in range(NIL, qtiles):
                m2 = m2p.tile([128, NSLOT], F32, tag="m2")
                for g in range(NGRPF):
                    main_group(qt, g, m2)
                main_leftover(qt, m2)
                main_tail(qt, m2)

    nc.compile()
    return nc


# --------------------------------------------------------------------------
# Dispatch B
# --------------------------------------------------------------------------

def build_dispatch_b(bq_slice, dbg=False):
    qtiles = bq_slice // 128
    nmemb = BSLOTS * 4            # 48 rescored members
    GCH = 6                       # slots per gather chunk tile

    nc = bacc.Bacc("TRN2", target_bir_lowering=False, debug=dbg)
    v_d = nc.dram_tensor("vals", [bq_slice, NCAND], F32,
                         kind="ExternalInput").ap()
    x_d = nc.dram_tensor("x", [bq_slice, D], F32, kind="ExternalInput").ap()
    k4_d = nc.dram_tensor("keysn4", [NQUAD, 4 * D], F32,
                          kind="ExternalInput").ap()
    val4_d = nc.dram_tensor("values4", [NPADR, D], F32,
                            kind="ExternalInput").ap()
    out_d = nc.dram_tensor("out", [bq_slice, D], F32, kind="ExternalOutput").ap()

    with tile.TileContext(nc) as tc:
        with (
            tc.tile_pool(name="const", bufs=1) as constp,
            tc.tile_pool(name="wp", bufs=2) as wp,
            tc.tile_pool(name="gp", bufs=2) as gp,
        ):
            # member-major tables for the 48 rescored entries: idx = m*12+s
            m_tab = constp.tile([128, nmemb], F32)      # member id m
            nc.gpsimd.iota(m_tab[:], pattern=[[1, 4], [0, BSLOTS]], base=0,
                           channel_multiplier=0,
                           allow_small_or_imprecise_dtypes=True)
            iota_m_f = constp.tile([128, nmemb], F32)   # 0..47
            nc.gpsimd.iota(iota_m_f[:], pattern=[[1, nmemb]], base=0,
                           channel_multiplier=0,
                           allow_small_or_imprecise_dtypes=True)

            for qt in range(qtiles):
                r0, r1 = qt * 128, (qt + 1) * 128

                # --- x_norm (exact fp32) ---
                xt = wp.tile([128, D], F32, tag="xt")
                nc.sync.dma_start(out=xt[:], in_=x_d[r0:r1, :])
                xsq = wp.tile([128, D], F32, tag="xsq")
                xn2 = wp.tile([128, 1], F32, tag="xn2")
                nc.scalar.activation(xsq[:], xt[:], ACTF.Square, accum_out=xn2[:])
                xsrt = wp.tile([128, 1], F32, tag="xsrt")
                nc.scalar.activation(xsrt[:], xn2[:], ACTF.Sqrt)
                xinv = wp.tile([128, 1], F32, tag="xinv")
                nc.vector.reciprocal(xinv[:], xsrt[:])
                xn = wp.tile([128, D], F32, tag="xn")
                nc.scalar.activation(xn[:], xt[:], ACTF.Copy, scale=xinv[:])

                # --- candidates: globally-unique packed (score | R) ---
                vin = wp.tile([128, NCAND], F32, tag="vin")
                nc.sync.dma_start(out=vin[:], in_=v_d[r0:r1, :])

                # --- prune to top-BSLOTS ---
                t12 = wp.tile([128, 16], F32, tag="t12")
                nc.vector.max(out=t12[:, 0:8], in_=vin[:])
                vrep = wp.tile([128, NCAND], F32, tag="vrep")
                nc.vector.match_replace(out=vrep[:], in_to_replace=t12[:, 0:8],
                                        in_values=vin[:], imm_value=_NEG_BIG)
                nc.vector.max(out=t12[:, 8:16], in_=vrep[:])

                # --- quad row R = low 15 bits of the packed value ---
                R_i = wp.tile([128, BSLOTS], I32, tag="ri")
                nc.vector.tensor_scalar(R_i[:], t12[:, :BSLOTS].bitcast(I32),
                                        _PACK_MASK, None, op0=ALU.bitwise_and)
                R_f = wp.tile([128, BSLOTS], F32, tag="rf")
                nc.vector.tensor_copy(R_f[:], R_i[:])

                # value rows rowv[m*12+s] = 4*R_s + m
                rowv = wp.tile([128, 4, BSLOTS], F32, tag="rowv")
                nc.vector.tensor_scalar(
                    rowv[:], R_f[:].unsqueeze(1).to_broadcast(
                        [128, 4, BSLOTS]),
                    4.0, None, op0=ALU.mult)
                nc.vector.tensor_tensor(
                    rowv[:].rearrange("p m s -> p (m s)"),
                    rowv[:].rearrange("p m s -> p (m s)"), m_tab[:],
                    op=ALU.add)

                # --- gather quad rows (chunked) + exact rescore ---
                sco = wp.tile([128, 4, BSLOTS], F32, tag="sco")
                xb = xn[:].unsqueeze(1).to_broadcast([128, GCH, D])
                for s0 in range(0, BSLOTS, GCH):
                    g = gp.tile([128, GCH, 4 * D], F32, tag=f"g{s0 // GCH}")
                    for s in range(GCH):
                        nc.gpsimd.indirect_dma_start(
                            out=g[:, s, :], out_offset=None, in_=k4_d[:],
                            in_offset=IndirectOffsetOnAxis(
                                ap=R_i[:, s0 + s:s0 + s + 1], axis=0))
                    gm = g[:].rearrange("p s (m d) -> p s m d", d=D)
                    for m in range(4):
                        prod = gp.tile([128, GCH, D], F32, tag=f"prod{m % 2}")
                        nc.vector.tensor_tensor(
                            prod[:], gm[:, :, m, :], xb, op=ALU.mult)
                        if m < 2:
                            nc.vector.tensor_reduce(
                                sco[:, m, s0:s0 + GCH], prod[:], axis=AX.X,
                                op=ALU.add)
                        else:
                            # per-slot reduction on the idle ACT engine
                            for s in range(GCH):
                                nc.scalar.activation(
                                    prod[:, s, :], prod[:, s, :], ACTF.Copy,
                                    accum_out=sco[:, m, s0 + s:s0 + s + 1])

                # --- exact top-8 of the 48 members ---
                scof = sco[:].rearrange("p m s -> p (m s)")
                top8 = wp.tile([128, 8], F32, tag="top8")
                nc.vector.max(out=top8[:], in_=scof)
                pos8 = wp.tile([128, 8], U32, tag="pos8")
                nc.vector.max_index(pos8[:], top8[:], scof)
                pos8f = wp.tile([128, 8], F32, tag="pos8f")
                nc.vector.tensor_copy(pos8f[:], pos8[:])

                # --- softmax ---
                sh = wp.tile([128, 8], F32, tag="sh")
                nc.vector.tensor_tensor(sh[:], top8[:],
                                        top8[:, 0:1].to_broadcast([128, 8]),
                                        op=ALU.subtract)
                ex = wp.tile([128, 8], F32, tag="ex")
                nc.scalar.activation(ex[:], sh[:], ACTF.Exp)
                es = wp.tile([128, 1], F32, tag="es")
                nc.vector.tensor_reduce(es[:], ex[:], axis=AX.X, op=ALU.add)
                esr = wp.tile([128, 1], F32, tag="esr")
                nc.vector.reciprocal(esr[:], es[:])
                wgt = wp.tile([128, 8], F32, tag="wgt")
                nc.vector.tensor_tensor(wgt[:], ex[:],
                                        esr[:].to_broadcast([128, 8]),
                                        op=ALU.mult)

                # --- winner value rows via one-hot over member index ---
                winr = wp.tile([128, 8], F32, tag="winr")
                ohm = wp.tile([128, nmemb], F32, tag="ohm")
                rowvf = rowv[:].rearrange("p m s -> p (m s)")
                for w in range(8):
                    nc.vector.scalar_tensor_tensor(
                        ohm[:], iota_m_f[:], pos8f[:, w:w + 1], rowvf,
                        op0=ALU.is_equal, op1=ALU.mult)
                    nc.vector.tensor_reduce(winr[:, w:w + 1], ohm[:], axis=AX.X,
                                            op=ALU.add)
                winr_i = wp.tile([128, 8], I32, tag="winri")
                nc.vector.tensor_copy(winr_i[:], winr[:])

                # --- gather value rows, weighted sum ---
                vg = gp.tile([128, 8, D], F32, tag="vg")
                for k in range(8):
                    nc.gpsimd.indirect_dma_start(
                        out=vg[:, k, :], out_offset=None, in_=val4_d[:],
                        in_offset=IndirectOffsetOnAxis(ap=winr_i[:, k:k + 1],
                                                       axis=0))
                vw = gp.tile([128, 8, D], F32, tag="vw")
                nc.vector.tensor_tensor(
                    vw[:], vg[:],
                    wgt[:].unsqueeze(2).to_broadcast([128, 8, D]), op=ALU.mult)
                ot = wp.tile([128, D], F32, tag="ot")
                nc.vector.tensor_reduce(ot[:], vw[:].rearrange("p k d -> p d k"),
                                        axis=AX.X, op=ALU.add)
                nc.sync.dma_start(out=out_d[r0:r1, :], in_=ot[:])

    nc.compile()
    return nc


# --------------------------------------------------------------------------
# Host orchestration
# --------------------------------------------------------------------------

_CACHE = {}


def _get_programs():
    if "A" not in _CACHE:
        _CACHE["A"] = build_dispatch_a()
    if "B" not in _CACHE:
        _CACHE["B"] = build_dispatch_b(B // NCORES)
    return _CACHE["A"], _CACHE["B"]


def _quad_cols():
    """P(S, m) per core; -1 for pad members."""
    P = -np.ones((NSLOT, 4), np.int64)
    S = np.arange(3072)
    G, j = S // 512, S % 512
    for m in range(4):
        P[S, m] = 2048 * G + 512 * m + j
    Sl = np.arange(3072, NSLOT)
    P[Sl, 0] = 12288 + (Sl - 3072)
    return P


_P4 = _quad_cols()                               # [NSLOT, 4]
_P4_VALID = _P4 >= 0
_P4G = (np.arange(NCORES)[:, None, None] * NLOC + _P4[None])  # [8, NSLOT, 4]


def kernel(x, keys, values, top_k):
    assert int(top_k) == TOPK
    x = np.ascontiguousarray(np.asarray(x, dtype=np.float32))
    keys = np.asarray(keys, dtype=np.float32)
    values = np.asarray(values, dtype=np.float32)
    assert x.shape == (B, D) and keys.shape == (N, D) and values.shape == (N, D)

    keys_sh = np.zeros((NCORES * NLOC, D), dtype=np.float32)
    keys_sh[:N] = keys
    values_sh = np.zeros((NCORES * NLOC, D), dtype=np.float32)
    values_sh[:N] = values

    nc_a, nc_b = _get_programs()
    core_ids = list(range(NCORES))

    # ---- dispatch A ----
    in_maps_a = [
        {"x": x,
         "keys": np.ascontiguousarray(keys_sh[c * NLOC:(c + 1) * NLOC]),
         "cbase": np.full((128, 1), c * NSLOT, dtype=np.int32)}
        for c in range(NCORES)
    ]
    t0 = time.perf_counter()
    res_a = run_bass_kernel_spmd(nc_a, in_maps_a, core_ids)
    t1 = time.perf_counter()
    cand = np.concatenate([res_a.results[c]["cand"] for c in range(NCORES)],
                          axis=1)  # [B, 64]

    # ---- host glue: normalized quad tables ----
    norms = np.sqrt(np.einsum("nd,nd->n", keys_sh, keys_sh))
    kn_sh = keys_sh / np.maximum(norms, 1e-12)[:, None]
    valid = np.broadcast_to(_P4_VALID[None], _P4G.shape)
    k4 = np.zeros((NCORES, NSLOT, 4, D), dtype=np.float32)
    k4[valid] = kn_sh[_P4G[valid]]
    k4 = np.ascontiguousarray(k4.reshape(NQUAD, 4 * D))
    v4 = np.zeros((NCORES, NSLOT, 4, D), dtype=np.float32)
    v4[valid] = values_sh[_P4G[valid]]
    v4 = np.ascontiguousarray(v4.reshape(NPADR, D))

    # ---- dispatch B ----
    bs = B // NCORES
    in_maps_b = [
        {
            "vals": np.ascontiguousarray(cand[c * bs:(c + 1) * bs]),
            "x": np.ascontiguousarray(x[c * bs:(c + 1) * bs]),
            "keysn4": k4,
            "values4": v4,
        }
        for c in range(NCORES)
    ]
    t2 = time.perf_counter()
    res_b = run_bass_kernel_spmd(nc_b, in_maps_b, core_ids)
    t3 = time.perf_counter()
    out = np.concatenate([res_b.results[c]["out"] for c in range(NCORES)],
                         axis=0)
    kernel.last_walltimes = (t1 - t0, t3 - t2)
    return out.astype(np.float32)


# revision 20
# speedup vs baseline: 1.4305x; 1.0682x over previous
"""Distributed cosine-similarity kNN retrieval (EpisodicSDM) on 8 Trainium2 cores.

Reference computation:
    x_norm = normalize(x); k_norm = normalize(keys)
    scores = x_norm @ k_norm.T               [B, N]
    top_vals, top_idx = top_k(scores, 8)
    out = sum_k softmax(top_vals)_k * values[top_idx_k]

Two SPMD dispatches, no collectives.

Dispatch A (keys sharded along N: 12544 columns/core, all queries):
  - key prep: row inv-norms (squares alternate ACT/DVE to balance engines),
    diag(1/||k||) built on GPSIMD, fp32 matmul transpose-and-scale ->
    kT bf16 [128, 2, 12544]
  - coarse scores in bf16 (fp32 PSUM accumulate), 512-wide matmuls
  - quad-slot reduction: 6 full groups of 2048 cols; within group,
    col P = 2048G + 512m + j belongs to slot S = 512G + j (member m).
    fold1 (DVE): max(pa, pb) pairs members (m, m+2); fold2 (DVE):
    max of m1 halves pairs member parity -> m2[S] = max over 4 members.
    Leftover cols 12288..12544 are single-member slots (ACT copy).
  - qtile 0's groups are interleaved with the key-prep blocks so the main
    pipeline starts as soon as kT block 0 exists.
  - pack slot id (12 bits, linear) into the f32 mantissa, max8 -> per-core
    top-8 packed (value, slot), tie-free.
  -> output: packed candidates [B, 8] per core.

Host glue: normalize keys (numpy), build the quad-layout rescore tables:
  keysn4[c*NSLOT + S] = 4 normalized member key rows (pads zero),
  values4[4*(c*NSLOT+S) + m] = member value rows.

Dispatch B (queries sharded, 256 per core):
  - re-pack candidates by position, top-12 slots of 64 via
    max8 + match_replace + max8
  - slot -> quad row R = NSLOT*(pos>>3) + pid; ONE indirect gather per slot
    fetches all 4 member rows (4096B contiguous)
  - exact fp32 rescore of 48 members via fused tensor_tensor_reduce,
    top-8, softmax, value rows 4R+m gathered, weighted sum.
"""

import sys
import time

_TRN_REPO = "/opt/trn_rl_repo"
if _TRN_REPO not in sys.path:
    sys.path.insert(0, _TRN_REPO)

import numpy as np

import concourse.bass as bass
import concourse.mybir as mybir
import concourse.tile as tile
from concourse import bacc
from concourse.bass import IndirectOffsetOnAxis
from concourse.bass_utils import run_bass_kernel_spmd
from concourse.masks import make_identity

F32 = mybir.dt.float32
BF16 = mybir.dt.bfloat16
I32 = mybir.dt.int32
U32 = mybir.dt.uint32
ALU = mybir.AluOpType
ACTF = mybir.ActivationFunctionType
AX = mybir.AxisListType

# ---- problem constants ----
B = 2048
D = 256
N = 100000
TOPK = 8
NCORES = 8
NLOC = 12544              # key columns per core; 8*12544 = 100352 >= N
NSLOT = 3328              # 6*512 quad slots + 256 single-member leftover
NGRPF = 6                 # full quad groups of 2048 columns
NQUAD = NCORES * NSLOT    # rescore-table quad rows
NPADR = 4 * NQUAD         # member rows in values4
BSLOTS = 12               # slots rescored per query after the merge
NCAND = NCORES * 8

_PACK_MASK = 0x7FFF
_NEG_BIG = -3.0e38


# --------------------------------------------------------------------------
# Dispatch A
# --------------------------------------------------------------------------

def build_dispatch_a(bq=B, dbg=False):
    qtiles = bq // 128
    kchunks = NLOC // 128          # 98

    nc = bacc.Bacc("TRN2", target_bir_lowering=False, debug=dbg)
    x_d = nc.dram_tensor("x", [bq, D], F32, kind="ExternalInput").ap()
    k_d = nc.dram_tensor("keys", [NLOC, D], F32, kind="ExternalInput").ap()
    cb_d = nc.dram_tensor("cbase", [128, 1], I32, kind="ExternalInput").ap()
    out_d = nc.dram_tensor("cand", [bq, 8], F32, kind="ExternalOutput").ap()

    with tile.TileContext(nc) as tc:
        with (
            tc.tile_pool(name="const", bufs=1) as constp,
            tc.tile_pool(name="kprep", bufs=4) as kprep,
            tc.tile_pool(name="big", bufs=1) as bigp,
            tc.tile_pool(name="xp", bufs=2) as xp,
            tc.tile_pool(name="sp", bufs=4) as sp,
            tc.tile_pool(name="m1p", bufs=2) as m1p,
            tc.tile_pool(name="m2p", bufs=2) as m2p,
            tc.tile_pool(name="ps", bufs=2, space="PSUM") as psp,
        ):
            identb = constp.tile([128, 128], BF16)
            make_identity(nc, identb[:])
            identf = constp.tile([128, 128], F32)
            make_identity(nc, identf[:])
            eps = constp.tile([128, 1], F32)
            nc.gpsimd.memset(eps[:], 1e-30)
            iota_pack = constp.tile([128, NSLOT], I32)
            nc.gpsimd.iota(iota_pack[:], pattern=[[1, NSLOT]], base=0,
                           channel_multiplier=0)
            cbase = constp.tile([128, 1], I32)
            nc.sync.dma_start(out=cbase[:], in_=cb_d[:, :])
            nc.gpsimd.tensor_tensor(iota_pack[:], iota_pack[:],
                                    cbase[:].to_broadcast([128, NSLOT]),
                                    op=ALU.add)
            maskc = constp.tile([128, 1], I32)
            nc.gpsimd.memset(maskc[:], -(_PACK_MASK + 1))

            kT = bigp.tile([128, 2, NLOC], BF16)
            xT = bigp.tile([128, 2, bq], BF16)

            # ---- x prep: normalize, cast, transpose into xT ----
            for qt in range(qtiles):
                xt = xp.tile([128, D], F32, tag="xt")
                nc.sync.dma_start(out=xt[:], in_=x_d[qt * 128:(qt + 1) * 128, :])
                xsq = xp.tile([128, D], F32, tag="xsq")
                nc.vector.tensor_tensor(xsq[:], xt[:], xt[:], op=ALU.mult)
                xn2 = xp.tile([128, 1], F32, tag="xn2")
                nc.vector.tensor_reduce(xn2[:], xsq[:], axis=AX.X, op=ALU.add)
                xsrt = xp.tile([128, 1], F32, tag="xsrt")
                nc.scalar.activation(xsrt[:], xn2[:], ACTF.Sqrt)
                xinv = xp.tile([128, 1], F32, tag="xinv")
                nc.vector.reciprocal(xinv[:], xsrt[:])
                xnb = xp.tile([128, D], BF16, tag="xnb")
                nc.scalar.activation(xnb[:], xt[:], ACTF.Copy, scale=xinv[:])
                pt = psp.tile([128, 1024], F32, tag="pa")
                for c in range(2):
                    nc.tensor.matmul(pt[:, c * 128:(c + 1) * 128],
                                     lhsT=xnb[:, c * 128:(c + 1) * 128],
                                     rhs=identb[:], start=True, stop=True)
                nc.scalar.activation(
                    xT[:, :, qt * 128:(qt + 1) * 128], pt[:, :256], ACTF.Copy)

            # ---- helpers ----
            def key_prep_chunks(chunks):
                """Prep a run of 128-key chunks (multiple of 4 or the tail)."""
                for g4s in range(0, len(chunks), 4):
                    grp = chunks[g4s:g4s + 4]
                    nch = len(grp)
                    ktfs = []
                    kn2g = kprep.tile([128, 4], F32, tag="kn2g")
                    for i, ch in enumerate(grp):
                        ktf = kprep.tile([128, D], F32, tag=f"ktf{i}")
                        nc.sync.dma_start(out=ktf[:],
                                          in_=k_d[ch * 128:(ch + 1) * 128, :])
                        if ch % 4 == 0:
                            ksq = kprep.tile([128, D], F32, tag=f"ksq{i % 2}")
                            nc.scalar.activation(ksq[:], ktf[:], ACTF.Square,
                                                 accum_out=kn2g[:, i:i + 1])
                        else:
                            ksq = kprep.tile([128, D], F32, tag=f"ksq{i % 2}")
                            nc.vector.tensor_tensor(ksq[:], ktf[:], ktf[:],
                                                    op=ALU.mult)
                            nc.vector.tensor_reduce(kn2g[:, i:i + 1], ksq[:],
                                                    axis=AX.X, op=ALU.add)
                        ktfs.append(ktf)
                    ksrt = kprep.tile([128, 4], F32, tag="ksrt")
                    nc.scalar.activation(ksrt[:, :nch], kn2g[:, :nch],
                                         ACTF.Sqrt, bias=eps[:])
                    kinvg = kprep.tile([128, 4], F32, tag="kinvg")
                    nc.vector.reciprocal(kinvg[:, :nch], ksrt[:, :nch])
                    for i, ch in enumerate(grp):
                        diag = kprep.tile([128, 128], F32, tag=f"diag{i}")
                        nc.gpsimd.tensor_tensor(
                            diag[:], identf[:],
                            kinvg[:, i:i + 1].to_broadcast([128, 128]),
                            op=ALU.mult)
                        pt = psp.tile([128, 1024], F32, tag="pb")
                        for c in range(2):
                            nc.tensor.matmul(
                                pt[:, c * 128:(c + 1) * 128],
                                lhsT=ktfs[i][:, c * 128:(c + 1) * 128],
                                rhs=diag[:], start=True, stop=True)
                        if ch % 4 == 1:
                            nc.vector.tensor_copy(
                                kT[:, :, ch * 128:(ch + 1) * 128],
                                pt[:, :256])
                        else:
                            nc.scalar.activation(
                                kT[:, :, ch * 128:(ch + 1) * 128],
                                pt[:, :256], ACTF.Copy)

            NBF = 6   # groups using bf16 folds (ACT copies both PSUM banks)

            def main_group(qt, g, m2):
                qs = slice(qt * 128, (qt + 1) * 128)
                base = 2048 * g
                pa = psp.tile([128, 1024], F32, tag="pa")
                pb = psp.tile([128, 1024], F32, tag="pb")
                for half, pp in ((0, pa), (1, pa), (2, pb), (3, pb)):
                    dst = pp[:, (half % 2) * 512:(half % 2 + 1) * 512]
                    cs = base + half * 512
                    for c in range(2):
                        nc.tensor.matmul(
                            dst, lhsT=xT[:, c, qs],
                            rhs=kT[:, c, cs:cs + 512],
                            start=(c == 0), stop=(c == 1))
                if g < NBF:
                    # bf16 path: ACT casts both banks, DVE folds at 2x
                    sa = sp.tile([128, 1024], BF16, tag="sa")
                    nc.scalar.activation(sa[:], pa[:], ACTF.Copy)
                    sb = sp.tile([128, 1024], BF16, tag="sb")
                    nc.scalar.activation(sb[:], pb[:], ACTF.Copy)
                    m1b = m1p.tile([128, 1024], BF16, tag="m1b")
                    nc.vector.tensor_tensor(m1b[:], sa[:], sb[:], op=ALU.max)
                    nc.vector.tensor_tensor(
                        m2[:, 512 * g:512 * (g + 1)],
                        m1b[:, :512], m1b[:, 512:], op=ALU.max)
                    # pack slot ids: bf16-widened f32 has low 16 bits zero,
                    # so an integer add == bitwise OR of the 15-bit id
                    nc.gpsimd.tensor_tensor(
                        m2[:, 512 * g:512 * (g + 1)].bitcast(I32),
                        m2[:, 512 * g:512 * (g + 1)].bitcast(I32),
                        iota_pack[:, 512 * g:512 * (g + 1)], op=ALU.add)
                else:
                    stmp = sp.tile([128, 1024], F32, tag="stmp")
                    nc.scalar.activation(stmp[:], pa[:], ACTF.Copy)
                    m1g = m1p.tile([128, 1024], F32, tag="m1g")
                    nc.vector.tensor_tensor(m1g[:], pb[:], stmp[:], op=ALU.max)
                    nc.vector.tensor_tensor(
                        m2[:, 512 * g:512 * (g + 1)],
                        m1g[:, :512], m1g[:, 512:], op=ALU.max)

            def main_leftover(qt, m2):
                qs = slice(qt * 128, (qt + 1) * 128)
                pa = psp.tile([128, 1024], F32, tag="pa")
                for c in range(2):
                    nc.tensor.matmul(pa[:, :256], lhsT=xT[:, c, qs],
                                     rhs=kT[:, c, 12288:12544],
                                     start=(c == 0), stop=(c == 1))
                nc.scalar.activation(m2[:, 3072:3328], pa[:, :256], ACTF.Copy)

            def main_tail(qt, m2):
                # leftover slots come from a full-precision f32 copy: mask+OR
                nc.vector.scalar_tensor_tensor(
                    m2[:, 3072:].bitcast(I32), m2[:, 3072:].bitcast(I32),
                    maskc[:], iota_pack[:, 3072:],
                    op0=ALU.bitwise_and, op1=ALU.bitwise_or)
                top = m2p.tile([128, 8], F32, tag="top")
                nc.vector.max(out=top[:], in_=m2[:])
                nc.sync.dma_start(out=out_d[qt * 128:(qt + 1) * 128, :],
                                  in_=top[:])

            # ---- qtiles 0..3 interleaved with key prep blocks ----
            NIL = 0
            m2s = []
            for _i in range(NIL):
                m2q = m2p.tile([128, NSLOT], F32, tag="m2")
                m2s.append(m2q)

            key_prep_chunks(list(range(0, 16)))
            for g in range(NGRPF):
                # overlap mains of ready block g with prep of block g+1
                nxt = (list(range(16 * (g + 1), 16 * (g + 2)))
                       if g + 1 < NGRPF else [96, 97])
                if NIL:
                    bsz = (len(nxt) + NIL - 1) // NIL
                    for q in range(NIL):
                        key_prep_chunks(nxt[bsz * q:bsz * (q + 1)])
                        main_group(q, g, m2s[q])
                else:
                    key_prep_chunks(nxt)
            for q in range(NIL):
                main_leftover(q, m2s[q])
                main_tail(q, m2s[q])

            # ---- remaining qtiles ----
            for qt in range(NIL, qtiles):
                m2 = m2p.tile([128, NSLOT], F32, tag="m2")
                for g in range(NGRPF):
                    main_group(qt, g, m2)
                main_leftover(qt, m2)
                main_tail(qt, m2)

    nc.compile()
    return nc


# --------------------------------------------------------------------------
# Dispatch B
# --------------------------------------------------------------------------

def build_dispatch_b(bq_slice, dbg=False):
    qtiles = bq_slice // 128
    nmemb = BSLOTS * 4            # 48 rescored members
    GCH = 3                       # slots per gather chunk tile

    nc = bacc.Bacc("TRN2", target_bir_lowering=False, debug=dbg)
    v_d = nc.dram_tensor("vals", [bq_slice, NCAND], F32,
                         kind="ExternalInput").ap()
    x_d = nc.dram_tensor("x", [bq_slice, D], F32, kind="ExternalInput").ap()
    k4_d = nc.dram_tensor("keysn4", [NQUAD, 4 * D], F32,
                          kind="ExternalInput").ap()
    val4_d = nc.dram_tensor("values4", [NPADR, D], F32,
                            kind="ExternalInput").ap()
    out_d = nc.dram_tensor("out", [bq_slice, D], F32, kind="ExternalOutput").ap()

    with tile.TileContext(nc) as tc:
        with (
            tc.tile_pool(name="const", bufs=1) as constp,
            tc.tile_pool(name="wp", bufs=2) as wp,
            tc.tile_pool(name="gp", bufs=2) as gp,
        ):
            # member-major tables for the 48 rescored entries: idx = m*12+s
            m_tab = constp.tile([128, nmemb], F32)      # member id m
            nc.gpsimd.iota(m_tab[:], pattern=[[1, 4], [0, BSLOTS]], base=0,
                           channel_multiplier=0,
                           allow_small_or_imprecise_dtypes=True)
            iota_m_f = constp.tile([128, nmemb], F32)   # 0..47
            nc.gpsimd.iota(iota_m_f[:], pattern=[[1, nmemb]], base=0,
                           channel_multiplier=0,
                           allow_small_or_imprecise_dtypes=True)

            wgts = {}
            winrs = {}
            for qt in range(qtiles):
                r0, r1 = qt * 128, (qt + 1) * 128

                # --- x_norm (exact fp32) ---
                xt = wp.tile([128, D], F32, tag="xt")
                nc.sync.dma_start(out=xt[:], in_=x_d[r0:r1, :])
                xsq = wp.tile([128, D], F32, tag="xsq")
                xn2 = wp.tile([128, 1], F32, tag="xn2")
                nc.scalar.activation(xsq[:], xt[:], ACTF.Square, accum_out=xn2[:])
                xsrt = wp.tile([128, 1], F32, tag="xsrt")
                nc.scalar.activation(xsrt[:], xn2[:], ACTF.Sqrt)
                xinv = wp.tile([128, 1], F32, tag="xinv")
                nc.vector.reciprocal(xinv[:], xsrt[:])
                xn = wp.tile([128, D], F32, tag="xn")
                nc.scalar.activation(xn[:], xt[:], ACTF.Copy, scale=xinv[:])

                # --- candidates: globally-unique packed (score | R) ---
                vin = wp.tile([128, NCAND], F32, tag="vin")
                nc.sync.dma_start(out=vin[:], in_=v_d[r0:r1, :])

                # --- prune to top-BSLOTS ---
                t12 = wp.tile([128, 16], F32, tag="t12")
                nc.vector.max(out=t12[:, 0:8], in_=vin[:])
                vrep = wp.tile([128, NCAND], F32, tag="vrep")
                nc.vector.match_replace(out=vrep[:], in_to_replace=t12[:, 0:8],
                                        in_values=vin[:], imm_value=_NEG_BIG)
                nc.vector.max(out=t12[:, 8:16], in_=vrep[:])

                # --- quad row R = low 15 bits of the packed value ---
                R_i = wp.tile([128, BSLOTS], I32, tag="ri")
                nc.vector.tensor_scalar(R_i[:], t12[:, :BSLOTS].bitcast(I32),
                                        _PACK_MASK, None, op0=ALU.bitwise_and)
                R_f = wp.tile([128, BSLOTS], F32, tag="rf")
                nc.vector.tensor_copy(R_f[:], R_i[:])

                # value rows rowv[m*12+s] = 4*R_s + m
                rowv = wp.tile([128, 4, BSLOTS], F32, tag="rowv")
                nc.vector.tensor_scalar(
                    rowv[:], R_f[:].unsqueeze(1).to_broadcast(
                        [128, 4, BSLOTS]),
                    4.0, None, op0=ALU.mult)
                nc.vector.tensor_tensor(
                    rowv[:].rearrange("p m s -> p (m s)"),
                    rowv[:].rearrange("p m s -> p (m s)"), m_tab[:],
                    op=ALU.add)

                # --- gather quad rows (chunked) + exact rescore ---
                sco = wp.tile([128, 4, BSLOTS], F32, tag="sco")
                xb = xn[:].unsqueeze(1).to_broadcast([128, GCH, D])
                for s0 in range(0, BSLOTS, GCH):
                    g = gp.tile([128, GCH, 4 * D], F32, tag=f"g{s0 // GCH}")
                    for s in range(GCH):
                        nc.gpsimd.indirect_dma_start(
                            out=g[:, s, :], out_offset=None, in_=k4_d[:],
                            in_offset=IndirectOffsetOnAxis(
                                ap=R_i[:, s0 + s:s0 + s + 1], axis=0))
                    gm = g[:].rearrange("p s (m d) -> p s m d", d=D)
                    for m in range(4):
                        prod = gp.tile([128, GCH, D], F32, tag=f"prod{m % 2}")
                        nc.vector.tensor_tensor(
                            prod[:], gm[:, :, m, :], xb, op=ALU.mult)
                        if m < 2:
                            nc.vector.tensor_reduce(
                                sco[:, m, s0:s0 + GCH], prod[:], axis=AX.X,
                                op=ALU.add)
                        else:
                            # per-slot reduction on the idle ACT engine
                            for s in range(GCH):
                                nc.scalar.activation(
                                    prod[:, s, :], prod[:, s, :], ACTF.Copy,
                                    accum_out=sco[:, m, s0 + s:s0 + s + 1])

                # --- exact top-8 of the 48 members ---
                scof = sco[:].rearrange("p m s -> p (m s)")
                top8 = wp.tile([128, 8], F32, tag="top8")
                nc.vector.max(out=top8[:], in_=scof)
                pos8 = wp.tile([128, 8], U32, tag="pos8")
                nc.vector.max_index(pos8[:], top8[:], scof)
                pos8f = wp.tile([128, 8], F32, tag="pos8f")
                nc.vector.tensor_copy(pos8f[:], pos8[:])

                # --- softmax ---
                sh = wp.tile([128, 8], F32, tag="sh")
                nc.vector.tensor_tensor(sh[:], top8[:],
                                        top8[:, 0:1].to_broadcast([128, 8]),
                                        op=ALU.subtract)
                ex = wp.tile([128, 8], F32, tag="ex")
                nc.scalar.activation(ex[:], sh[:], ACTF.Exp)
                es = wp.tile([128, 1], F32, tag="es")
                nc.vector.tensor_reduce(es[:], ex[:], axis=AX.X, op=ALU.add)
                esr = wp.tile([128, 1], F32, tag="esr")
                nc.vector.reciprocal(esr[:], es[:])
                wgt = wp.tile([128, 8], F32, tag="wgt")
                nc.vector.tensor_tensor(wgt[:], ex[:],
                                        esr[:].to_broadcast([128, 8]),
                                        op=ALU.mult)

                # --- winner value rows via one-hot over member index ---
                winr = wp.tile([128, 8], F32, tag="winr")
                ohm = wp.tile([128, nmemb], F32, tag="ohm")
                rowvf = rowv[:].rearrange("p m s -> p (m s)")
                for w in range(8):
                    nc.vector.scalar_tensor_tensor(
                        ohm[:], iota_m_f[:], pos8f[:, w:w + 1], rowvf,
                        op0=ALU.is_equal, op1=ALU.mult)
                    nc.vector.tensor_reduce(winr[:, w:w + 1], ohm[:], axis=AX.X,
                                            op=ALU.add)
                winr_i = wp.tile([128, 8], I32, tag="winri")
                nc.vector.tensor_copy(winr_i[:], winr[:])
                wgts[qt] = wgt
                winrs[qt] = winr_i

            # --- phase 2: value gathers + weighted sum (after all key work) ---
            for qt in range(qtiles):
                r0, r1 = qt * 128, (qt + 1) * 128
                wgt = wgts[qt]
                winr_i = winrs[qt]
                vg = gp.tile([128, 8, D], F32, tag="vg")
                for k in range(8):
                    nc.gpsimd.indirect_dma_start(
                        out=vg[:, k, :], out_offset=None, in_=val4_d[:],
                        in_offset=IndirectOffsetOnAxis(ap=winr_i[:, k:k + 1],
                                                       axis=0))
                vw = gp.tile([128, 8, D], F32, tag="vw")
                nc.vector.tensor_tensor(
                    vw[:], vg[:],
                    wgt[:].unsqueeze(2).to_broadcast([128, 8, D]), op=ALU.mult)
                ot = wp.tile([128, D], F32, tag="ot")
                nc.vector.tensor_reduce(ot[:], vw[:].rearrange("p k d -> p d k"),
                                        axis=AX.X, op=ALU.add)
                nc.sync.dma_start(out=out_d[r0:r1, :], in_=ot[:])

    nc.compile()
    return nc


# --------------------------------------------------------------------------
# Host orchestration
# --------------------------------------------------------------------------

_CACHE = {}


def _get_programs():
    if "A" not in _CACHE:
        _CACHE["A"] = build_dispatch_a()
    if "B" not in _CACHE:
        _CACHE["B"] = build_dispatch_b(B // NCORES)
    return _CACHE["A"], _CACHE["B"]


def _quad_cols():
    """P(S, m) per core; -1 for pad members."""
    P = -np.ones((NSLOT, 4), np.int64)
    S = np.arange(3072)
    G, j = S // 512, S % 512
    for m in range(4):
        P[S, m] = 2048 * G + 512 * m + j
    Sl = np.arange(3072, NSLOT)
    P[Sl, 0] = 12288 + (Sl - 3072)
    return P


_P4 = _quad_cols()                               # [NSLOT, 4]
_P4_VALID = _P4 >= 0
_P4G = (np.arange(NCORES)[:, None, None] * NLOC + _P4[None])  # [8, NSLOT, 4]


def kernel(x, keys, values, top_k):
    assert int(top_k) == TOPK
    x = np.ascontiguousarray(np.asarray(x, dtype=np.float32))
    keys = np.asarray(keys, dtype=np.float32)
    values = np.asarray(values, dtype=np.float32)
    assert x.shape == (B, D) and keys.shape == (N, D) and values.shape == (N, D)

    keys_sh = np.zeros((NCORES * NLOC, D), dtype=np.float32)
    keys_sh[:N] = keys
    values_sh = np.zeros((NCORES * NLOC, D), dtype=np.float32)
    values_sh[:N] = values

    nc_a, nc_b = _get_programs()
    core_ids = list(range(NCORES))

    # ---- dispatch A ----
    in_maps_a = [
        {"x": x,
         "keys": np.ascontiguousarray(keys_sh[c * NLOC:(c + 1) * NLOC]),
         "cbase": np.full((128, 1), c * NSLOT, dtype=np.int32)}
        for c in range(NCORES)
    ]
    t0 = time.perf_counter()
    res_a = run_bass_kernel_spmd(nc_a, in_maps_a, core_ids)
    t1 = time.perf_counter()
    cand = np.concatenate([res_a.results[c]["cand"] for c in range(NCORES)],
                          axis=1)  # [B, 64]

    # ---- host glue: normalized quad tables ----
    norms = np.sqrt(np.einsum("nd,nd->n", keys_sh, keys_sh))
    kn_sh = keys_sh / np.maximum(norms, 1e-12)[:, None]
    valid = np.broadcast_to(_P4_VALID[None], _P4G.shape)
    k4 = np.zeros((NCORES, NSLOT, 4, D), dtype=np.float32)
    k4[valid] = kn_sh[_P4G[valid]]
    k4 = np.ascontiguousarray(k4.reshape(NQUAD, 4 * D))
    v4 = np.zeros((NCORES, NSLOT, 4, D), dtype=np.float32)
    v4[valid] = values_sh[_P4G[valid]]
    v4 = np.ascontiguousarray(v4.reshape(NPADR, D))

    # ---- dispatch B ----
    bs = B // NCORES
    in_maps_b = [
        {
            "vals": np.ascontiguousarray(cand[c * bs:(c + 1) * bs]),
            "x": np.ascontiguousarray(x[c * bs:(c + 1) * bs]),
            "keysn4": k4,
            "values4": v4,
        }
        for c in range(NCORES)
    ]
    t2 = time.perf_counter()
    res_b = run_bass_kernel_spmd(nc_b, in_maps_b, core_ids)
    t3 = time.perf_counter()
    out = np.concatenate([res_b.results[c]["out"] for c in range(NCORES)],
                         axis=0)
    kernel.last_walltimes = (t1 - t0, t3 - t2)
    return out.astype(np.float32)


# revision 25
# speedup vs baseline: 1.4400x; 1.0066x over previous
"""Distributed cosine-similarity kNN retrieval (EpisodicSDM) on 8 Trainium2 cores.

Reference computation:
    x_norm = normalize(x); k_norm = normalize(keys)
    scores = x_norm @ k_norm.T               [B, N]
    top_vals, top_idx = top_k(scores, 8)
    out = sum_k softmax(top_vals)_k * values[top_idx_k]

Two SPMD dispatches, no collectives.

Dispatch A (keys sharded along N: 12544 columns/core, all queries):
  - key prep: row inv-norms (squares alternate ACT/DVE to balance engines),
    diag(1/||k||) built on GPSIMD, fp32 matmul transpose-and-scale ->
    kT bf16 [128, 2, 12544]
  - coarse scores in bf16 (fp32 PSUM accumulate), 512-wide matmuls
  - quad-slot reduction: 6 full groups of 2048 cols; within group,
    col P = 2048G + 512m + j belongs to slot S = 512G + j (member m).
    fold1 (DVE): max(pa, pb) pairs members (m, m+2); fold2 (DVE):
    max of m1 halves pairs member parity -> m2[S] = max over 4 members.
    Leftover cols 12288..12544 are single-member slots (ACT copy).
  - qtile 0's groups are interleaved with the key-prep blocks so the main
    pipeline starts as soon as kT block 0 exists.
  - pack slot id (12 bits, linear) into the f32 mantissa, max8 -> per-core
    top-8 packed (value, slot), tie-free.
  -> output: packed candidates [B, 8] per core.

Host glue: normalize keys (numpy), build the quad-layout rescore tables:
  keysn4[c*NSLOT + S] = 4 normalized member key rows (pads zero),
  values4[4*(c*NSLOT+S) + m] = member value rows.

Dispatch B (queries sharded, 256 per core):
  - re-pack candidates by position, top-12 slots of 64 via
    max8 + match_replace + max8
  - slot -> quad row R = NSLOT*(pos>>3) + pid; ONE indirect gather per slot
    fetches all 4 member rows (4096B contiguous)
  - exact fp32 rescore of 48 members via fused tensor_tensor_reduce,
    top-8, softmax, value rows 4R+m gathered, weighted sum.
"""

import sys
import time

_TRN_REPO = "/opt/trn_rl_repo"
if _TRN_REPO not in sys.path:
    sys.path.insert(0, _TRN_REPO)

import ml_dtypes
import numpy as np

import concourse.bass as bass
import concourse.mybir as mybir
import concourse.tile as tile
from concourse import bacc
from concourse.bass import IndirectOffsetOnAxis
from concourse.bass_utils import run_bass_kernel_spmd
from concourse.masks import make_identity

F32 = mybir.dt.float32
BF16 = mybir.dt.bfloat16
I32 = mybir.dt.int32
U32 = mybir.dt.uint32
ALU = mybir.AluOpType
ACTF = mybir.ActivationFunctionType
AX = mybir.AxisListType

# ---- problem constants ----
B = 2048
D = 256
N = 100000
TOPK = 8
NCORES = 8
NLOC = 12544              # key columns per core; 8*12544 = 100352 >= N
NSLOT = 3328              # 6*512 quad slots + 256 single-member leftover
NGRPF = 6                 # full quad groups of 2048 columns
NQUAD = NCORES * NSLOT    # rescore-table quad rows
NPADR = 4 * NQUAD         # member rows in values4
BSLOTS = 12               # slots rescored per query after the merge
NCAND = NCORES * 8

_PACK_MASK = 0x7FFF
_NEG_BIG = -3.0e38


# --------------------------------------------------------------------------
# Dispatch A
# --------------------------------------------------------------------------

def build_dispatch_a(bq=B, dbg=False):
    qtiles = bq // 128
    kchunks = NLOC // 128          # 98

    nc = bacc.Bacc("TRN2", target_bir_lowering=False, debug=dbg)
    x_d = nc.dram_tensor("x", [bq, D], F32, kind="ExternalInput").ap()
    k_d = nc.dram_tensor("keys", [NLOC, D], BF16, kind="ExternalInput").ap()
    cb_d = nc.dram_tensor("cbase", [128, 1], I32, kind="ExternalInput").ap()
    out_d = nc.dram_tensor("cand", [bq, 8], F32, kind="ExternalOutput").ap()

    with tile.TileContext(nc) as tc:
        with (
            tc.tile_pool(name="const", bufs=1) as constp,
            tc.tile_pool(name="kprep", bufs=4) as kprep,
            tc.tile_pool(name="big", bufs=1) as bigp,
            tc.tile_pool(name="xp", bufs=2) as xp,
            tc.tile_pool(name="sp", bufs=4) as sp,
            tc.tile_pool(name="m1p", bufs=2) as m1p,
            tc.tile_pool(name="m2p", bufs=2) as m2p,
            tc.tile_pool(name="ps", bufs=2, space="PSUM") as psp,
        ):
            identb = constp.tile([128, 128], BF16)
            make_identity(nc, identb[:])
            identf = constp.tile([128, 128], F32)
            make_identity(nc, identf[:])
            eps = constp.tile([128, 1], F32)
            nc.gpsimd.memset(eps[:], 1e-30)
            iota_pack = constp.tile([128, NSLOT], I32)
            nc.gpsimd.iota(iota_pack[:], pattern=[[1, NSLOT]], base=0,
                           channel_multiplier=0)
            cbase = constp.tile([128, 1], I32)
            nc.sync.dma_start(out=cbase[:], in_=cb_d[:, :])
            nc.gpsimd.tensor_tensor(iota_pack[:], iota_pack[:],
                                    cbase[:].to_broadcast([128, NSLOT]),
                                    op=ALU.add)
            maskc = constp.tile([128, 1], I32)
            nc.gpsimd.memset(maskc[:], -(_PACK_MASK + 1))

            kT = bigp.tile([128, 2, NLOC], BF16)
            xT = bigp.tile([128, 2, bq], BF16)

            # ---- x prep: normalize, cast, transpose into xT ----
            for qt in range(qtiles):
                xt = xp.tile([128, D], F32, tag="xt")
                nc.sync.dma_start(out=xt[:], in_=x_d[qt * 128:(qt + 1) * 128, :])
                xsq = xp.tile([128, D], F32, tag="xsq")
                nc.vector.tensor_tensor(xsq[:], xt[:], xt[:], op=ALU.mult)
                xn2 = xp.tile([128, 1], F32, tag="xn2")
                nc.vector.tensor_reduce(xn2[:], xsq[:], axis=AX.X, op=ALU.add)
                xsrt = xp.tile([128, 1], F32, tag="xsrt")
                nc.scalar.activation(xsrt[:], xn2[:], ACTF.Sqrt)
                xinv = xp.tile([128, 1], F32, tag="xinv")
                nc.vector.reciprocal(xinv[:], xsrt[:])
                xnb = xp.tile([128, D], BF16, tag="xnb")
                nc.scalar.activation(xnb[:], xt[:], ACTF.Copy, scale=xinv[:])
                pt = psp.tile([128, 1024], F32, tag="pa")
                for c in range(2):
                    nc.tensor.matmul(pt[:, c * 128:(c + 1) * 128],
                                     lhsT=xnb[:, c * 128:(c + 1) * 128],
                                     rhs=identb[:], start=True, stop=True)
                nc.scalar.activation(
                    xT[:, :, qt * 128:(qt + 1) * 128], pt[:, :256], ACTF.Copy)

            # ---- helpers ----
            def key_prep_chunks(chunks):
                """Prep a run of 128-key chunks (multiple of 4 or the tail)."""
                for g4s in range(0, len(chunks), 4):
                    grp = chunks[g4s:g4s + 4]
                    nch = len(grp)
                    ktfs = []
                    kn2g = kprep.tile([128, 4], F32, tag="kn2g")
                    for i, ch in enumerate(grp):
                        ktf = kprep.tile([128, D], BF16, tag=f"ktf{i}")
                        nc.sync.dma_start(out=ktf[:],
                                          in_=k_d[ch * 128:(ch + 1) * 128, :])
                        if ch % 4 == 0:
                            ksq = kprep.tile([128, D], F32, tag=f"ksq{i % 2}")
                            nc.scalar.activation(ksq[:], ktf[:], ACTF.Square,
                                                 accum_out=kn2g[:, i:i + 1])
                        else:
                            ksq = kprep.tile([128, D], F32, tag=f"ksq{i % 2}")
                            nc.vector.tensor_tensor(ksq[:], ktf[:], ktf[:],
                                                    op=ALU.mult)
                            nc.vector.tensor_reduce(kn2g[:, i:i + 1], ksq[:],
                                                    axis=AX.X, op=ALU.add)
                        ktfs.append(ktf)
                    ksrt = kprep.tile([128, 4], F32, tag="ksrt")
                    nc.scalar.activation(ksrt[:, :nch], kn2g[:, :nch],
                                         ACTF.Sqrt, bias=eps[:])
                    kinvg = kprep.tile([128, 4], F32, tag="kinvg")
                    nc.vector.reciprocal(kinvg[:, :nch], ksrt[:, :nch])
                    for i, ch in enumerate(grp):
                        diag = kprep.tile([128, 128], BF16, tag=f"diag{i}")
                        nc.gpsimd.tensor_tensor(
                            diag[:], identb[:],
                            kinvg[:, i:i + 1].to_broadcast([128, 128]),
                            op=ALU.mult)
                        pt = psp.tile([128, 1024], F32, tag="pb")
                        for c in range(2):
                            nc.tensor.matmul(
                                pt[:, c * 128:(c + 1) * 128],
                                lhsT=ktfs[i][:, c * 128:(c + 1) * 128],
                                rhs=diag[:], start=True, stop=True)
                        if ch % 4 == 1:
                            nc.vector.tensor_copy(
                                kT[:, :, ch * 128:(ch + 1) * 128],
                                pt[:, :256])
                        else:
                            nc.scalar.activation(
                                kT[:, :, ch * 128:(ch + 1) * 128],
                                pt[:, :256], ACTF.Copy)

            NBF = 6   # groups using bf16 folds (ACT copies both PSUM banks)

            def main_group(qt, g, m2):
                qs = slice(qt * 128, (qt + 1) * 128)
                base = 2048 * g
                pa = psp.tile([128, 1024], F32, tag="pa")
                pb = psp.tile([128, 1024], F32, tag="pb")
                for half, pp in ((0, pa), (1, pa), (2, pb), (3, pb)):
                    dst = pp[:, (half % 2) * 512:(half % 2 + 1) * 512]
                    cs = base + half * 512
                    for c in range(2):
                        nc.tensor.matmul(
                            dst, lhsT=xT[:, c, qs],
                            rhs=kT[:, c, cs:cs + 512],
                            start=(c == 0), stop=(c == 1))
                if g < NBF:
                    # bf16 path: ACT casts both banks, DVE folds at 2x
                    sa = sp.tile([128, 1024], BF16, tag="sa")
                    nc.scalar.activation(sa[:], pa[:], ACTF.Copy)
                    sb = sp.tile([128, 1024], BF16, tag="sb")
                    nc.scalar.activation(sb[:], pb[:], ACTF.Copy)
                    m1b = m1p.tile([128, 1024], BF16, tag="m1b")
                    nc.vector.tensor_tensor(m1b[:], sa[:], sb[:], op=ALU.max)
                    nc.vector.tensor_tensor(
                        m2[:, 512 * g:512 * (g + 1)],
                        m1b[:, :512], m1b[:, 512:], op=ALU.max)
                    # pack slot ids: bf16-widened f32 has low 16 bits zero,
                    # so an integer add == bitwise OR of the 15-bit id
                    nc.gpsimd.tensor_tensor(
                        m2[:, 512 * g:512 * (g + 1)].bitcast(I32),
                        m2[:, 512 * g:512 * (g + 1)].bitcast(I32),
                        iota_pack[:, 512 * g:512 * (g + 1)], op=ALU.add)
                else:
                    stmp = sp.tile([128, 1024], F32, tag="stmp")
                    nc.scalar.activation(stmp[:], pa[:], ACTF.Copy)
                    m1g = m1p.tile([128, 1024], F32, tag="m1g")
                    nc.vector.tensor_tensor(m1g[:], pb[:], stmp[:], op=ALU.max)
                    nc.vector.tensor_tensor(
                        m2[:, 512 * g:512 * (g + 1)],
                        m1g[:, :512], m1g[:, 512:], op=ALU.max)

            def main_leftover(qt, m2):
                qs = slice(qt * 128, (qt + 1) * 128)
                pa = psp.tile([128, 1024], F32, tag="pa")
                for c in range(2):
                    nc.tensor.matmul(pa[:, :256], lhsT=xT[:, c, qs],
                                     rhs=kT[:, c, 12288:12544],
                                     start=(c == 0), stop=(c == 1))
                nc.scalar.activation(m2[:, 3072:3328], pa[:, :256], ACTF.Copy)

            def main_tail(qt, m2):
                # leftover slots come from a full-precision f32 copy: mask+OR
                nc.vector.scalar_tensor_tensor(
                    m2[:, 3072:].bitcast(I32), m2[:, 3072:].bitcast(I32),
                    maskc[:], iota_pack[:, 3072:],
                    op0=ALU.bitwise_and, op1=ALU.bitwise_or)
                top = m2p.tile([128, 8], F32, tag="top")
                nc.vector.max(out=top[:], in_=m2[:])
                nc.sync.dma_start(out=out_d[qt * 128:(qt + 1) * 128, :],
                                  in_=top[:])

            # ---- qtiles 0..3 interleaved with key prep blocks ----
            NIL = 0
            m2s = []
            for _i in range(NIL):
                m2q = m2p.tile([128, NSLOT], F32, tag="m2")
                m2s.append(m2q)

            key_prep_chunks(list(range(0, 16)))
            for g in range(NGRPF):
                # overlap mains of ready block g with prep of block g+1
                nxt = (list(range(16 * (g + 1), 16 * (g + 2)))
                       if g + 1 < NGRPF else [96, 97])
                if NIL:
                    bsz = (len(nxt) + NIL - 1) // NIL
                    for q in range(NIL):
                        key_prep_chunks(nxt[bsz * q:bsz * (q + 1)])
                        main_group(q, g, m2s[q])
                else:
                    key_prep_chunks(nxt)
            for q in range(NIL):
                main_leftover(q, m2s[q])
                main_tail(q, m2s[q])

            # ---- remaining qtiles ----
            for qt in range(NIL, qtiles):
                m2 = m2p.tile([128, NSLOT], F32, tag="m2")
                for g in range(NGRPF):
                    main_group(qt, g, m2)
                main_leftover(qt, m2)
                main_tail(qt, m2)

    nc.compile()
    return nc


# --------------------------------------------------------------------------
# Dispatch B
# --------------------------------------------------------------------------

def build_dispatch_b(bq_slice, dbg=False):
    qtiles = bq_slice // 128
    nmemb = BSLOTS * 4            # 48 rescored members
    GCH = 3                       # slots per gather chunk tile

    nc = bacc.Bacc("TRN2", target_bir_lowering=False, debug=dbg)
    v_d = nc.dram_tensor("vals", [bq_slice, NCAND], F32,
                         kind="ExternalInput").ap()
    x_d = nc.dram_tensor("x", [bq_slice, D], F32, kind="ExternalInput").ap()
    k4_d = nc.dram_tensor("keysn4", [NQUAD, 4 * D], F32,
                          kind="ExternalInput").ap()
    val4_d = nc.dram_tensor("values4", [NPADR, D], F32,
                            kind="ExternalInput").ap()
    out_d = nc.dram_tensor("out", [bq_slice, D], F32, kind="ExternalOutput").ap()

    with tile.TileContext(nc) as tc:
        with (
            tc.tile_pool(name="const", bufs=1) as constp,
            tc.tile_pool(name="wp", bufs=2) as wp,
            tc.tile_pool(name="gp", bufs=2) as gp,
        ):
            # member-major tables for the 48 rescored entries: idx = m*12+s
            m_tab = constp.tile([128, nmemb], F32)      # member id m
            nc.gpsimd.iota(m_tab[:], pattern=[[1, 4], [0, BSLOTS]], base=0,
                           channel_multiplier=0,
                           allow_small_or_imprecise_dtypes=True)
            iota_m_f = constp.tile([128, nmemb], F32)   # 0..47
            nc.gpsimd.iota(iota_m_f[:], pattern=[[1, nmemb]], base=0,
                           channel_multiplier=0,
                           allow_small_or_imprecise_dtypes=True)

            wgts = {}
            winrs = {}
            for qt in range(qtiles):
                r0, r1 = qt * 128, (qt + 1) * 128

                # --- x_norm (exact fp32) ---
                xt = wp.tile([128, D], F32, tag="xt")
                nc.sync.dma_start(out=xt[:], in_=x_d[r0:r1, :])
                xsq = wp.tile([128, D], F32, tag="xsq")
                xn2 = wp.tile([128, 1], F32, tag="xn2")
                nc.scalar.activation(xsq[:], xt[:], ACTF.Square, accum_out=xn2[:])
                xsrt = wp.tile([128, 1], F32, tag="xsrt")
                nc.scalar.activation(xsrt[:], xn2[:], ACTF.Sqrt)
                xinv = wp.tile([128, 1], F32, tag="xinv")
                nc.vector.reciprocal(xinv[:], xsrt[:])
                xn = wp.tile([128, D], F32, tag="xn")
                nc.scalar.activation(xn[:], xt[:], ACTF.Copy, scale=xinv[:])

                # --- candidates: globally-unique packed (score | R) ---
                vin = wp.tile([128, NCAND], F32, tag="vin")
                nc.sync.dma_start(out=vin[:], in_=v_d[r0:r1, :])

                # --- prune to top-BSLOTS ---
                t12 = wp.tile([128, 16], F32, tag="t12")
                nc.vector.max(out=t12[:, 0:8], in_=vin[:])
                vrep = wp.tile([128, NCAND], F32, tag="vrep")
                nc.vector.match_replace(out=vrep[:], in_to_replace=t12[:, 0:8],
                                        in_values=vin[:], imm_value=_NEG_BIG)
                nc.vector.max(out=t12[:, 8:16], in_=vrep[:])

                # --- quad row R = low 15 bits of the packed value ---
                R_i = wp.tile([128, BSLOTS], I32, tag="ri")
                nc.vector.tensor_scalar(R_i[:], t12[:, :BSLOTS].bitcast(I32),
                                        _PACK_MASK, None, op0=ALU.bitwise_and)
                R_f = wp.tile([128, BSLOTS], F32, tag="rf")
                nc.vector.tensor_copy(R_f[:], R_i[:])

                # value rows rowv[m*12+s] = 4*R_s + m
                rowv = wp.tile([128, 4, BSLOTS], F32, tag="rowv")
                nc.vector.tensor_scalar(
                    rowv[:], R_f[:].unsqueeze(1).to_broadcast(
                        [128, 4, BSLOTS]),
                    4.0, None, op0=ALU.mult)
                nc.vector.tensor_tensor(
                    rowv[:].rearrange("p m s -> p (m s)"),
                    rowv[:].rearrange("p m s -> p (m s)"), m_tab[:],
                    op=ALU.add)

                # --- gather quad rows (chunked) + exact rescore ---
                sco = wp.tile([128, 4, BSLOTS], F32, tag="sco")
                xb = xn[:].unsqueeze(1).to_broadcast([128, GCH, D])
                for s0 in range(0, BSLOTS, GCH):
                    g = gp.tile([128, GCH, 4 * D], F32, tag=f"g{s0 // GCH}")
                    for s in range(GCH):
                        nc.gpsimd.indirect_dma_start(
                            out=g[:, s, :], out_offset=None, in_=k4_d[:],
                            in_offset=IndirectOffsetOnAxis(
                                ap=R_i[:, s0 + s:s0 + s + 1], axis=0))
                    gm = g[:].rearrange("p s (m d) -> p s m d", d=D)
                    for m in range(4):
                        prod = gp.tile([128, GCH, D], F32, tag=f"prod{m % 2}")
                        nc.vector.tensor_tensor(
                            prod[:], gm[:, :, m, :], xb, op=ALU.mult)
                        if m < 2:
                            nc.vector.tensor_reduce(
                                sco[:, m, s0:s0 + GCH], prod[:], axis=AX.X,
                                op=ALU.add)
                        else:
                            # per-slot reduction on the idle ACT engine
                            for s in range(GCH):
                                nc.scalar.activation(
                                    prod[:, s, :], prod[:, s, :], ACTF.Copy,
                                    accum_out=sco[:, m, s0 + s:s0 + s + 1])

                # --- exact top-8 of the 48 members ---
                scof = sco[:].rearrange("p m s -> p (m s)")
                top8 = wp.tile([128, 8], F32, tag="top8")
                nc.vector.max(out=top8[:], in_=scof)
                pos8 = wp.tile([128, 8], U32, tag="pos8")
                nc.vector.max_index(pos8[:], top8[:], scof)
                pos8f = wp.tile([128, 8], F32, tag="pos8f")
                nc.vector.tensor_copy(pos8f[:], pos8[:])

                # --- softmax ---
                sh = wp.tile([128, 8], F32, tag="sh")
                nc.vector.tensor_tensor(sh[:], top8[:],
                                        top8[:, 0:1].to_broadcast([128, 8]),
                                        op=ALU.subtract)
                ex = wp.tile([128, 8], F32, tag="ex")
                nc.scalar.activation(ex[:], sh[:], ACTF.Exp)
                es = wp.tile([128, 1], F32, tag="es")
                nc.vector.tensor_reduce(es[:], ex[:], axis=AX.X, op=ALU.add)
                esr = wp.tile([128, 1], F32, tag="esr")
                nc.vector.reciprocal(esr[:], es[:])
                wgt = wp.tile([128, 8], F32, tag="wgt")
                nc.vector.tensor_tensor(wgt[:], ex[:],
                                        esr[:].to_broadcast([128, 8]),
                                        op=ALU.mult)

                # --- winner value rows via one-hot over member index ---
                winr = wp.tile([128, 8], F32, tag="winr")
                ohm = wp.tile([128, nmemb], F32, tag="ohm")
                rowvf = rowv[:].rearrange("p m s -> p (m s)")
                for w in range(8):
                    nc.vector.scalar_tensor_tensor(
                        ohm[:], iota_m_f[:], pos8f[:, w:w + 1], rowvf,
                        op0=ALU.is_equal, op1=ALU.mult)
                    nc.vector.tensor_reduce(winr[:, w:w + 1], ohm[:], axis=AX.X,
                                            op=ALU.add)
                winr_i = wp.tile([128, 8], I32, tag="winri")
                nc.vector.tensor_copy(winr_i[:], winr[:])
                wgts[qt] = wgt
                winrs[qt] = winr_i

            # --- phase 2: value gathers + weighted sum (after all key work) ---
            for qt in range(qtiles):
                r0, r1 = qt * 128, (qt + 1) * 128
                wgt = wgts[qt]
                winr_i = winrs[qt]
                vg = gp.tile([128, 8, D], F32, tag="vg")
                for k in range(8):
                    nc.gpsimd.indirect_dma_start(
                        out=vg[:, k, :], out_offset=None, in_=val4_d[:],
                        in_offset=IndirectOffsetOnAxis(ap=winr_i[:, k:k + 1],
                                                       axis=0))
                vw = gp.tile([128, 8, D], F32, tag="vw")
                nc.vector.tensor_tensor(
                    vw[:], vg[:],
                    wgt[:].unsqueeze(2).to_broadcast([128, 8, D]), op=ALU.mult)
                ot = wp.tile([128, D], F32, tag="ot")
                nc.vector.tensor_reduce(ot[:], vw[:].rearrange("p k d -> p d k"),
                                        axis=AX.X, op=ALU.add)
                nc.sync.dma_start(out=out_d[r0:r1, :], in_=ot[:])

    nc.compile()
    return nc


# --------------------------------------------------------------------------
# Host orchestration
# --------------------------------------------------------------------------

_CACHE = {}


def _get_programs():
    if "A" not in _CACHE:
        _CACHE["A"] = build_dispatch_a()
    if "B" not in _CACHE:
        _CACHE["B"] = build_dispatch_b(B // NCORES)
    return _CACHE["A"], _CACHE["B"]


def _quad_cols():
    """P(S, m) per core; -1 for pad members."""
    P = -np.ones((NSLOT, 4), np.int64)
    S = np.arange(3072)
    G, j = S // 512, S % 512
    for m in range(4):
        P[S, m] = 2048 * G + 512 * m + j
    Sl = np.arange(3072, NSLOT)
    P[Sl, 0] = 12288 + (Sl - 3072)
    return P


_P4 = _quad_cols()                               # [NSLOT, 4]
_P4_VALID = _P4 >= 0
_P4G = (np.arange(NCORES)[:, None, None] * NLOC + _P4[None])  # [8, NSLOT, 4]


def kernel(x, keys, values, top_k):
    assert int(top_k) == TOPK
    x = np.ascontiguousarray(np.asarray(x, dtype=np.float32))
    keys = np.asarray(keys, dtype=np.float32)
    values = np.asarray(values, dtype=np.float32)
    assert x.shape == (B, D) and keys.shape == (N, D) and values.shape == (N, D)

    keys_sh = np.zeros((NCORES * NLOC, D), dtype=np.float32)
    keys_sh[:N] = keys
    values_sh = np.zeros((NCORES * NLOC, D), dtype=np.float32)
    values_sh[:N] = values

    nc_a, nc_b = _get_programs()
    core_ids = list(range(NCORES))

    # ---- dispatch A ----
    in_maps_a = [
        {"x": x,
         "keys": np.ascontiguousarray(
             keys_sh[c * NLOC:(c + 1) * NLOC].astype(ml_dtypes.bfloat16)),
         "cbase": np.full((128, 1), c * NSLOT, dtype=np.int32)}
        for c in range(NCORES)
    ]
    t0 = time.perf_counter()
    res_a = run_bass_kernel_spmd(nc_a, in_maps_a, core_ids)
    t1 = time.perf_counter()
    cand = np.concatenate([res_a.results[c]["cand"] for c in range(NCORES)],
                          axis=1)  # [B, 64]

    # ---- host glue: normalized quad tables ----
    norms = np.sqrt(np.einsum("nd,nd->n", keys_sh, keys_sh))
    kn_sh = keys_sh / np.maximum(norms, 1e-12)[:, None]
    valid = np.broadcast_to(_P4_VALID[None], _P4G.shape)
    k4 = np.zeros((NCORES, NSLOT, 4, D), dtype=np.float32)
    k4[valid] = kn_sh[_P4G[valid]]
    k4 = np.ascontiguousarray(k4.reshape(NQUAD, 4 * D))
    v4 = np.zeros((NCORES, NSLOT, 4, D), dtype=np.float32)
    v4[valid] = values_sh[_P4G[valid]]
    v4 = np.ascontiguousarray(v4.reshape(NPADR, D))

    # ---- dispatch B ----
    bs = B // NCORES
    in_maps_b = [
        {
            "vals": np.ascontiguousarray(cand[c * bs:(c + 1) * bs]),
            "x": np.ascontiguousarray(x[c * bs:(c + 1) * bs]),
            "keysn4": k4,
            "values4": v4,
        }
        for c in range(NCORES)
    ]
    t2 = time.perf_counter()
    res_b = run_bass_kernel_spmd(nc_b, in_maps_b, core_ids)
    t3 = time.perf_counter()
    out = np.concatenate([res_b.results[c]["out"] for c in range(NCORES)],
                         axis=0)
    kernel.last_walltimes = (t1 - t0, t3 - t2)
    return out.astype(np.float32)
